# revision 34
# baseline (speedup 1.0000x reference)
"""Trainium2 Bass kernel for nn_GAT_WLN (GNN message passing, 8 NeuronCores).

Strategy (graph/data parallel per the sharding hint):
  - Nodes sharded 512/core; edges sharded by destination node into 128-node
    windows; one edge stream (real edges only, window-padded) shared by both
    message-passing phases. GAT self-loops are handled by dedicated per-
    window tiles that read the core's local node data — no gather, and they
    run inside the AllGather hole.
  - Input-linear edge/node encodings precomputed on host (same category as
    the baseline's h0/P precompute): Z = P[src] + ea@W1b.T + b1 (relu'd on
    device), SPg = ea@W2c.T + b2c. Z/SP stream through small rotating SBUF
    tiles. This removes all phase-B indirect gathers.
  - Per-window drains emit h1 node-major in one matmul chain (bias via a
    ones-row matmul), then R|g|a_s(hi/lo split, near-f32 exact) are shipped
    bf16 through one AllGather ([N, 516] table, 4x smaller than fp32 R|g).
  - Phase C gathers [128, 516] bf16 rows per edge tile (SWDGE); attention
    softmax without max-subtraction (validated |e| small).
  - q is allgathered (tiny, bf16).
  - Pairwise map q[x]+q[y]: 2 column-chunks on the PE via interleave
    matmuls vs a static eye pattern (drained on ACT), 6 chunks via one-wide
    DVE broadcast adds vs a PSUM-broadcast q row. Output written bf16 (host
    converts to f32). Diagonal -1 rows via data-driven indirect scatter.
  - PE HAM clock gate tripped to 2.4 GHz with junk matmul blocks at start
    and again before the pairwise phase (it cools during AG3).
"""
import numpy as np
import ml_dtypes

N, E = 4096, 32768
F, D, H, C = 82, 6, 256, 5
SLOPE = 0.2
NCORES = 8
NPC = N // NCORES          # 512 nodes per core
WIN = 128                  # dst window
WPC = NPC // WIN           # 4 windows per core
AG2W = 516                 # allgathered node payload width (bf16)

BF16 = ml_dtypes.bfloat16

_cache = {}


# ----------------------------------------------------------------------------
# host-side preprocessing
# ----------------------------------------------------------------------------
def _prep(edge_index, edge_attr, g):
    src = np.asarray(edge_index[0], dtype=np.int64)
    dst = np.asarray(edge_index[1], dtype=np.int64)
    ea = np.asarray(edge_attr, dtype=np.float32)

    order = np.argsort(dst, kind="stable")
    srcs, dsts = src[order], dst[order]
    eas = ea[order]

    groups = [[None] * WPC for _ in range(NCORES)]
    gidx = dsts // WIN
    bounds = np.searchsorted(gidx, np.arange(NCORES * WPC + 1))
    cnt = np.zeros((NCORES, WPC), np.int64)
    for r in range(NCORES):
        for w in range(WPC):
            lo, hi = bounds[r * WPC + w], bounds[r * WPC + w + 1]
            groups[r][w] = (lo, hi)
            cnt[r, w] = hi - lo

    T_w = int(-(-cnt.max() // 128))              # edge tiles per window
    T_tot = WPC * T_w

    # host input encodings (input-linear, same category as h0/P)
    f32 = np.float32
    x = np.asarray(g["x"], f32)
    h0f = np.maximum(x @ np.asarray(g["W_lin"], f32).T, 0.0)
    W1a = np.asarray(g["wl1_W1"], f32)[:, :H]
    W1b = np.asarray(g["wl1_W1"], f32)[:, H:]
    P_f32 = h0f @ W1a.T                                     # [N, H]
    qe_all = eas @ W1b.T + np.asarray(g["wl1_b1"], f32)     # [E, H]
    Zrows = (P_f32[srcs] + qe_all).astype(BF16)             # [E, H]
    sp_all = (eas @ np.asarray(g["wl2_W2"], f32).T
              + np.asarray(g["wl2_b2"], f32)).astype(BF16)  # [E, H]

    cores = []
    for r in range(NCORES):
        Z_sb = np.zeros((128, T_tot * H), BF16)
        SP_sb = np.zeros((128, T_tot * H), BF16)
        src_sb = np.zeros((128, T_tot), np.int32)
        ohBC = np.zeros((128, T_tot * 128), np.float32)
        ohGT = np.zeros((128, T_tot * 128), np.float32)
        Z3 = Z_sb.reshape(128, T_tot, H)
        SP3 = SP_sb.reshape(128, T_tot, H)
        for w in range(WPC):
            lo, hi = groups[r][w]
            nloc = (dsts[lo:hi] % WIN).astype(np.int64)
            pos = w * T_w * 128 + np.arange(hi - lo)
            tt, cc = pos // 128, pos % 128
            Z3[cc, tt] = Zrows[lo:hi]
            SP3[cc, tt] = sp_all[lo:hi]
            src_sb[cc, tt] = srcs[lo:hi]
            ohBC[cc, tt * 128 + nloc] = 1.0
            ohGT[nloc, tt * 128 + cc] = 1.0
        iloc = np.arange(NPC)
        diag_sb = ((iloc * N) + (r * NPC + iloc)).astype(np.int32) \
            .reshape(WPC, 128).T
        cores.append(dict(
            diag_sb=np.ascontiguousarray(diag_sb),
            Z_sb=Z_sb,
            SP_sb=SP_sb,
            src_sb=src_sb,
            ohBC=ohBC.astype(BF16),
            ohGATT=ohGT.astype(BF16),
            h0Tl=np.ascontiguousarray(
                h0f[r * NPC:(r + 1) * NPC].T.reshape(2, 128, NPC)
                .transpose(1, 0, 2).astype(BF16)),
        ))
    return cores, T_w


def _prep_weights(g):
    f32 = np.float32

    def kchunks(wT, nk, extra=None):
        # wT: [K, M] -> [128, nk, M(+1)] chunked along K; extra: [K] column
        K, M = wT.shape
        assert K == nk * 128
        w = np.asarray(wT, f32)
        if extra is not None:
            w = np.concatenate([w, np.asarray(extra, f32)[:, None]], axis=1)
        return np.ascontiguousarray(
            w.reshape(nk, 128, -1).transpose(1, 0, 2).astype(BF16))

    gat_W = np.asarray(g["gat_W"], f32)
    v_as = gat_W.T @ np.asarray(g["gat_asrc"], f32)   # [H]: a_s = h1 @ v_as
    v_ad = gat_W.T @ np.asarray(g["gat_adst"], f32)   # [H]: a_d = h1 @ v_ad

    out = {}
    out["w2T"] = kchunks(np.asarray(g["wl1_W2"], f32).T, 4)      # [128,4,256]
    out["b2row"] = np.asarray(g["wl1_b2"], f32)[None, :].astype(BF16)
    out["w3v"] = kchunks(np.asarray(g["wl2_W3"], f32).T, 2, v_as)  # [128,2,257]
    b3r = np.zeros((1, H + 1), f32)
    b3r[0, :H] = np.asarray(g["wl2_b3"], f32)
    out["b3row"] = b3r.astype(BF16)                               # [1,257]
    out["gatwv"] = kchunks(gat_W.T, 2, v_ad)                      # [128,2,257]
    out["wl2T"] = kchunks(np.asarray(g["W_lin2"], f32).T, 2)
    out["wl3T"] = kchunks(np.asarray(g["W_lin3"], f32).T, 2)
    out["b3c"] = np.ascontiguousarray(
        np.asarray(g["wl2_b3"], f32).reshape(2, 128).T)
    out["qconstc"] = np.ascontiguousarray(
        (((np.asarray(g["gat_b"], f32) @ np.asarray(g["W_lin2"], f32).T)
          @ np.asarray(g["W_lin3"], f32).T)[:, None]).astype(f32))
    out["pat5"] = np.ascontiguousarray(
        np.tile(np.eye(C, dtype=f32), N).astype(BF16))
    return out


# ----------------------------------------------------------------------------
# device program
# ----------------------------------------------------------------------------
def _build(T_w):
    import concourse.bass as bass
    import concourse.tile as tile
    from concourse import bacc, mybir
    from concourse.bass import IndirectOffsetOnAxis, ts, broadcast_tensor_aps
    from concourse.bass import _add_dep_helper as add_dep
    from concourse.masks import make_identity
    from contextlib import ExitStack

    f32 = mybir.dt.float32
    bf16 = mybir.dt.bfloat16
    i32 = mybir.dt.int32
    AF = mybir.ActivationFunctionType
    OP = mybir.AluOpType

    T_tot = WPC * T_w
    JCH = 512 * C          # 2560 output cols per chunk
    NJC = N // 512         # 8 chunks per row-tile

    nc = bacc.Bacc("TRN2", target_bir_lowering=False, debug=False,
                   enable_asserts=False, num_devices=NCORES)

    def inp(name, shape, dt=bf16):
        return nc.dram_tensor(name, list(shape), dt, kind="ExternalInput").ap()

    d_Z = inp("Z_sb", [128, T_tot * H])
    d_SP = inp("SP_sb", [128, T_tot * H])
    d_src = inp("src_sb", [128, T_tot], i32)
    d_ohBC = inp("ohBC", [128, T_tot * 128])
    d_ohGT = inp("ohGATT", [128, T_tot * 128])
    d_h0Tl = inp("h0Tl", [128, 2, NPC])
    d_w2T = inp("w2T", [128, 4, H])
    d_b2row = inp("b2row", [1, H])
    d_w3v = inp("w3v", [128, 2, H + 1])
    d_b3row = inp("b3row", [1, H + 1])
    d_gatwv = inp("gatwv", [128, 2, H + 1])
    d_wl2T = inp("wl2T", [128, 2, H])
    d_wl3T = inp("wl3T", [128, 2, C])
    d_b3c = inp("b3c", [128, 2], f32)
    d_qconstc = inp("qconstc", [C, 1], f32)
    d_diag = inp("diag_sb", [128, WPC], i32)
    d_pat5 = inp("pat5", [C, C * N])

    out_h = nc.dram_tensor("out", [NPC * N, C], bf16, kind="ExternalOutput")
    out_flat = out_h.ap()
    out2 = out_flat.rearrange("(i j) c -> i (j c)", i=NPC)

    with tile.TileContext(nc) as tc, ExitStack() as ctx:
        const = ctx.enter_context(tc.tile_pool(name="const", bufs=1))
        nodes = ctx.enter_context(tc.tile_pool(name="nodes", bufs=1))
        epool = ctx.enter_context(tc.tile_pool(name="edge", bufs=3))
        pwpool = ctx.enter_context(tc.tile_pool(name="pw", bufs=1))
        psum = ctx.enter_context(tc.tile_pool(name="psum", bufs=1, space="PSUM"))
        dram = ctx.enter_context(tc.tile_pool(name="dram", bufs=1, space="DRAM"))

        _n = [0]

        def pt(shape, tag="mm", dt=f32, bufs=4):
            _n[0] += 1
            return psum.tile(list(shape), dt, tag=tag, bufs=bufs,
                             name=f"ps{_n[0]}")

        def cload(name, ap, dt=bf16):
            t = const.tile(list(ap.shape), dt, name=name)
            nc.sync.dma_start(out=t[:], in_=ap)
            return t

        # collective buffers
        ag2_in = dram.tile([NPC, AG2W], bf16)
        ag2_out = dram.tile([N, AG2W], bf16, addr_space="Shared")
        ag3_in = dram.tile([NPC, C], bf16)
        ag3_out = dram.tile([N, C], bf16, addr_space="Shared")
        RG = [list(range(NCORES))]

        # phase-B-critical loads only; everything phase C needs is queued
        # after the phase B loop so its Z stream isn't stuck behind them
        NCH = 4
        sb_ohBC = const.tile([128, T_tot * 128], bf16, name="sb_ohBC")
        bc = (T_tot * 128) // NCH
        for ch in range(NCH):
            nc.sync.dma_start(out=sb_ohBC[:, ch * bc:(ch + 1) * bc],
                              in_=d_ohBC[:, ch * bc:(ch + 1) * bc])
        sb_w2T = cload("sb_w2T", d_w2T)
        sb_b2row = cload("sb_b2row", d_b2row)
        h0Tl = cload("h0Tl", d_h0Tl)
        sb_w3v = cload("sb_w3v", d_w3v)
        sb_b3row = cload("sb_b3row", d_b3row)
        sb_gatwv = cload("sb_gatwv", d_gatwv)
        identity = const.tile([128, 128], bf16)
        make_identity(nc, identity[:])
        identity_f = const.tile([128, 128], f32)
        make_identity(nc, identity_f[:])
        ones1 = const.tile([1, 128], bf16)
        nc.vector.memset(ones1[:], 1.0)

        # PE warm-up: ~6us of back-to-back matmuls trips the HAM clock gate
        # to 2.4 GHz before real work arrives (PE is otherwise idle here)
        warm_rhs = const.tile([128, 512], bf16)
        nc.vector.memset(warm_rhs[:], 0.0)
        for _ in range(10):
            p = pt([128, 512], tag="aggG", bufs=2)
            nc.tensor.matmul(p[:], lhsT=identity[:], rhs=warm_rhs[:],
                             start=True, stop=True)

        def transpose_128(dst_ap, src_ap):
            p = pt([src_ap.shape[1], src_ap.shape[0]], dt=bf16)
            nc.tensor.transpose(p[:], src_ap,
                                identity[:src_ap.shape[0], :src_ap.shape[0]])
            nc.vector.tensor_copy(dst_ap, p[:])

        # ========== phase B: relu(Z) -> agg -> h1 -> R|g|a_s per window =====
        agg_nm = nodes.tile([128, WPC, H], bf16)
        aggT = nodes.tile([128, 2, NPC], bf16)
        h1_nm = nodes.tile([128, WPC, H], bf16)
        h1T = nodes.tile([128, 2, NPC], bf16)
        ag2row = nodes.tile([128, WPC, 514], bf16)
        ad_bf = nodes.tile([128, WPC], bf16)
        ad_f32 = nodes.tile([128, WPC], f32)
        aggp = [None] * WPC
        for t in range(T_tot):
            w = t // T_w
            if t % T_w == 0:
                aggp[w] = pt([128, H], tag="agg", bufs=2)
            zt = epool.tile([128, H], bf16, tag="zin", bufs=8)
            nc.sync.dma_start(out=zt[:], in_=d_Z[:, ts(t, H)])
            msg = epool.tile([128, H], bf16, tag="msg")
            nc.scalar.activation(msg[:], zt[:], AF.Relu)
            nc.tensor.matmul(aggp[w][:], lhsT=sb_ohBC[:, ts(t, 128)],
                             rhs=msg[:],
                             start=(t % T_w == 0), stop=(t % T_w == T_w - 1),
                             skip_group_check=True)
            if t % T_w != T_w - 1:
                continue
            # ---- window w drained: h1 -> R|g|a_s -> AG2 input rows ----
            wsl = ts(w, 128)
            nc.scalar.copy(agg_nm[:, w, :], aggp[w][:])
            for m in range(2):
                transpose_128(aggT[:, m, wsl], agg_nm[:, w, ts(m, 128)])
            ph = pt([128, H])
            for kc in range(4):
                lhs = aggT[:, kc, wsl] if kc < 2 else h0Tl[:, kc - 2, wsl]
                nc.tensor.matmul(ph[:], lhsT=lhs, rhs=sb_w2T[:, kc, :],
                                 start=(kc == 0), stop=False)
            nc.tensor.matmul(ph[:], lhsT=ones1[:], rhs=sb_b2row[:],
                             start=False, stop=True)
            nc.scalar.activation(h1_nm[:, w, :], ph[:], AF.Relu)
            for m in range(2):
                transpose_128(h1T[:, m, wsl], h1_nm[:, w, ts(m, 128)])
            pr = pt([128, H + 1], tag="agg", bufs=2)
            for kc in range(2):
                nc.tensor.matmul(pr[:], lhsT=h1T[:, kc, wsl],
                                 rhs=sb_w3v[:, kc, :],
                                 start=(kc == 0), stop=False)
            nc.tensor.matmul(pr[:], lhsT=ones1[:], rhs=sb_b3row[:],
                             start=False, stop=True)
            nc.scalar.copy(ag2row[:, w, 0:H], pr[:, 0:H])
            nc.vector.tensor_copy(ag2row[:, w, 512:513], pr[:, H:H + 1])
            nc.vector.tensor_tensor(ag2row[:, w, 513:514], pr[:, H:H + 1],
                                    ag2row[:, w, 512:513], op=OP.subtract)
            pg = pt([128, H + 1], tag="agg", bufs=2)
            for kc in range(2):
                nc.tensor.matmul(pg[:], lhsT=h1T[:, kc, wsl],
                                 rhs=sb_gatwv[:, kc, :],
                                 start=(kc == 0), stop=(kc == 1))
            nc.scalar.copy(ag2row[:, w, H:2 * H], pg[:, 0:H])
            nc.vector.tensor_copy(ad_bf[:, w:w + 1], pg[:, H:H + 1])
            nc.vector.tensor_copy(ad_f32[:, w:w + 1], pg[:, H:H + 1])
            nc.sync.dma_start(out=ag2_in[wsl, 0:514], in_=ag2row[:, w, :])

        nc.gpsimd.collective_compute("AllGather", OP.bypass, replica_groups=RG,
                                     ins=[ag2_in.opt()], outs=[ag2_out.opt()])

        # phase-C / tail const loads (queued behind phase B's Z stream)
        sb_src = cload("sb_src", d_src, i32)
        sb_ohGT = cload("sb_ohGT", d_ohGT)
        sb_wl2T = cload("sb_wl2T", d_wl2T)
        sb_wl3T = cload("sb_wl3T", d_wl3T)
        sb_b3c = cload("sb_b3c", d_b3c, f32)
        sb_qconst = cload("sb_qconst", d_qconstc, f32)
        sb_diag = cload("sb_diag", d_diag, i32)
        neg1 = const.tile([128, C], bf16)
        nc.vector.memset(neg1[:], -1.0)

        # ========== self-loop GAT tiles: local data only, run in the
        # collective hole (no gather, no WL-output contribution) ==========
        aggS_g = nodes.tile([128, WPC, H + 1], f32)
        for w in range(WPC):
            tas = epool.tile([128, 1], f32, tag="tas")
            nc.vector.scalar_tensor_tensor(tas[:], in0=ag2row[:, w, 512:513],
                                           scalar=1.0,
                                           in1=ag2row[:, w, 513:514],
                                           op0=OP.mult, op1=OP.add)
            eatt = epool.tile([128, 1], f32, tag="eatt")
            nc.scalar.activation(eatt[:], tas[:], AF.Identity,
                                 bias=ad_f32[:, w:w + 1])
            el = epool.tile([128, 1], f32, tag="el")
            nc.vector.scalar_tensor_tensor(el[:], in0=eatt[:], scalar=SLOPE,
                                           in1=eatt[:], op0=OP.mult,
                                           op1=OP.max)
            ex = epool.tile([128, 1], f32, tag="ex")
            nc.scalar.activation(ex[:], el[:], AF.Exp)
            wmsg = epool.tile([128, H + 1], bf16, tag="wmsg")
            nc.scalar.activation(wmsg[:, 0:H], ag2row[:, w, H:2 * H], AF.Copy,
                                 scale=ex[:])
            nc.scalar.copy(wmsg[:, H:H + 1], ex[:])
            ps = pt([128, H + 1], tag="aggG", bufs=2)
            nc.tensor.matmul(ps[:], lhsT=identity[:], rhs=wmsg[:],
                             start=True, stop=True)
            nc.scalar.copy(aggS_g[:, w, :], ps[:])

        # a_d per edge — no AG2 dependency, also fills the collective hole
        ad_e_all = nodes.tile([128, T_tot], f32)
        for t in range(T_tot):
            w = t // T_w
            pd = pt([128, 1])
            nc.tensor.matmul(pd[:], lhsT=sb_ohGT[:, ts(t, 128)],
                             rhs=ad_bf[:, w:w + 1], start=True, stop=True)
            nc.vector.tensor_copy(ad_e_all[:, t:t + 1], pd[:])

        # ========== phase C gathered edges ================================
        u_nm = nodes.tile([128, WPC, H], bf16, tag="nmA2")
        glob_nm = nodes.tile([128, WPC, H], bf16, tag="nmB2")
        uT = nodes.tile([128, 2, NPC], bf16, tag="ftA")
        globT = nodes.tile([128, 2, NPC], bf16, tag="ftB")
        preT = nodes.tile([128, 2, NPC], bf16)
        t1T = nodes.tile([128, 2, NPC], bf16)
        qsb = nodes.tile([C, NPC], f32)
        q_nm = nodes.tile([128, WPC, C], bf16)
        aggcp = [None] * WPC
        agggp = [None] * WPC
        for t in range(T_tot):
            w = t // T_w
            k = t % T_w
            if k == 0:
                aggcp[w] = pt([128, H], tag="agg", bufs=2)
                agggp[w] = pt([128, H + 1], tag="aggG", bufs=2)
            gR = epool.tile([128, AG2W], bf16, tag="gath2", bufs=8)
            nc.gpsimd.indirect_dma_start(
                out=gR[:], out_offset=None, in_=ag2_out[:, :],
                in_offset=IndirectOffsetOnAxis(ap=sb_src[:, t:t + 1], axis=0))
            spt = epool.tile([128, H], bf16, tag="spin", bufs=8)
            nc.sync.dma_start(out=spt[:], in_=d_SP[:, ts(t, H)])
            msg2 = epool.tile([128, H], bf16, tag="msg")
            nc.vector.tensor_tensor(msg2[:], gR[:, 0:H], spt[:], op=OP.mult)
            nc.tensor.matmul(aggcp[w][:], lhsT=sb_ohBC[:, ts(t, 128)],
                             rhs=msg2[:],
                             start=(k == 0), stop=(k == T_w - 1),
                             skip_group_check=True)
            tas = epool.tile([128, 1], f32, tag="tas")
            nc.vector.scalar_tensor_tensor(tas[:], in0=gR[:, 512:513],
                                           scalar=1.0, in1=gR[:, 513:514],
                                           op0=OP.mult, op1=OP.add)
            eatt = epool.tile([128, 1], f32, tag="eatt")
            nc.scalar.activation(eatt[:], tas[:], AF.Identity,
                                 bias=ad_e_all[:, t:t + 1])
            el = epool.tile([128, 1], f32, tag="el")
            nc.vector.scalar_tensor_tensor(el[:], in0=eatt[:], scalar=SLOPE,
                                           in1=eatt[:], op0=OP.mult,
                                           op1=OP.max)
            ex = epool.tile([128, 1], f32, tag="ex")
            nc.scalar.activation(ex[:], el[:], AF.Exp)
            wmsg = epool.tile([128, H + 1], bf16, tag="wmsg")
            nc.scalar.activation(wmsg[:, 0:H], gR[:, H:2 * H], AF.Copy,
                                 scale=ex[:])
            nc.scalar.copy(wmsg[:, H:H + 1], ex[:])
            nc.tensor.matmul(agggp[w][:], lhsT=sb_ohBC[:, ts(t, 128)],
                             rhs=wmsg[:],
                             start=(k == 0), stop=(k == T_w - 1),
                             skip_group_check=True)
            if k != T_w - 1:
                continue
            # ---- window complete: combine with self partials ----
            nc.vector.tensor_mul(u_nm[:, w, :], aggcp[w][:], h1_nm[:, w, :])
            tmpg = epool.tile([128, H + 1], f32, tag="tmpg", bufs=2)
            nc.vector.tensor_add(tmpg[:], agggp[w][:], aggS_g[:, w, :])
            rec = epool.tile([128, 1], f32, tag="rec")
            nc.vector.reciprocal(rec[:], tmpg[:, H:H + 1])
            nc.vector.tensor_scalar(glob_nm[:, w, :], tmpg[:, 0:H],
                                    rec[:], None, op0=OP.mult)

        # ========== tail: q (per-window slices, emitted post-loop so the
        # scheduler runs w0-2 during remaining phase-C gathers) ==========
        for w in range(WPC):
            wsl = ts(w, 128)
            for m in range(2):
                transpose_128(uT[:, m, wsl], u_nm[:, w, ts(m, 128)])
                transpose_128(globT[:, m, wsl], glob_nm[:, w, ts(m, 128)])
            for m in range(2):
                p = pt([128, 128])
                for kc in range(2):
                    nc.tensor.matmul(p[:], lhsT=sb_w3v[:, kc, ts(m, 128)],
                                     rhs=uT[:, kc, wsl],
                                     start=(kc == 0), stop=(kc == 1))
                lt = epool.tile([128, 128], bf16, tag="loc", bufs=2)
                nc.scalar.activation(lt[:], p[:], AF.Identity,
                                     bias=sb_b3c[:, m:m + 1])
                nc.vector.tensor_add(preT[:, m, wsl], lt[:], globT[:, m, wsl])
            for m in range(2):
                p = pt([128, 128])
                for kc in range(2):
                    nc.tensor.matmul(p[:], lhsT=sb_wl2T[:, kc, ts(m, 128)],
                                     rhs=preT[:, kc, wsl],
                                     start=(kc == 0), stop=(kc == 1))
                nc.scalar.copy(t1T[:, m, wsl], p[:])
            qp5 = pt([C, 128])
            for kc in range(2):
                nc.tensor.matmul(qp5[:], lhsT=sb_wl3T[:, kc, :],
                                 rhs=t1T[:, kc, wsl],
                                 start=(kc == 0), stop=(kc == 1))
            nc.vector.tensor_scalar(qsb[:, wsl], qp5[:], sb_qconst[:], None,
                                    op0=OP.add)
            pq = pt([128, C])
            nc.tensor.transpose(pq[:], qsb[:, wsl], identity_f[:C, :C])
            nc.vector.tensor_copy(q_nm[:, w, :], pq[:])
            nc.sync.dma_start(out=ag3_in[wsl, :], in_=q_nm[:, w, :])

        nc.gpsimd.collective_compute("AllGather", OP.bypass, replica_groups=RG,
                                     ins=[ag3_in.opt()], outs=[ag3_out.opt()])

        # ========== pairwise map =====
        # patt row 0: q[j,c] flattened (base-0: matmul rhs + bcast source);
        # rows 1-5: static eye interleave for the PE-matmul ocs
        patt = nodes.tile([C + 1, C * N], bf16, tag="bigbuf")
        nc.sync.dma_start(out=patt[1:C + 1, :], in_=d_pat5)
        ag3o_flat = ag3_out[:, :].rearrange("n c -> (n c)")[None, :]
        nc.sync.dma_start(out=patt[0:1, :], in_=ag3o_flat)
        patt5 = patt[0:1, :]

        # lhsT for the PE ocs: row 0 = 1 (q[j] term), rows 1-5 = local q —
        # rows 1-5 written via casting SWDGE DMA (engine ops need base-0)
        lhsTq = pwpool.tile([C + 1, NPC], bf16)
        nc.vector.memset(lhsTq[:], 1.0)
        nc.gpsimd.dma_start(out=lhsTq[1:C + 1, :], in_=qsb[:])

        # re-warm the PE right at pairwise start (it cooled during AG3);
        # gating the rhs on the AG3-loaded patt row pins these to the front
        # of the pairwise phase as one contiguous >=3.4us busy block
        for _ in range(8):
            p = pt([128, 512], tag="aggG", bufs=2)
            nc.tensor.matmul(p[:], lhsT=ones1[:], rhs=patt5[:, 0:512],
                             start=True, stop=True)

        pw_tags = ["mm", "agg", "aggG", "mm", "agg"]
        pw_bufs = {"mm": 4, "agg": 2, "aggG": 2}
        N_PE_OC = 2            # ocs on PE via interleave matmul; rest on DVE
        slab_dmas = [[] for _ in range(WPC)]
        for oc in range(NJC):
            if oc >= N_PE_OC:
                qbc = pwpool.tile([128, JCH], bf16, tag="qbc", bufs=2,
                                  name=f"qbc{oc}")
                for s in range(C):
                    tag = pw_tags[s]
                    p = psum.tile([128, 512], f32, tag=tag, bufs=pw_bufs[tag],
                                  name=f"pwp{oc}_{s}")
                    nc.tensor.matmul(p[:], lhsT=ones1[:],
                                     rhs=patt5[:, oc * JCH + s * 512:
                                               oc * JCH + (s + 1) * 512],
                                     start=True, stop=True)
                    nc.scalar.copy(qbc[:, ts(s, 512)], p[:])
                qbc3 = qbc[:].rearrange("p (j c) -> p j c", c=C)
            for it in range(WPC):
                ot = pwpool.tile([128, JCH], bf16, tag="ot", bufs=6,
                                 name=f"ot{oc}_{it}")
                if oc >= N_PE_OC:
                    ot3 = ot[:].rearrange("p (j c) -> p j c", c=C)
                    qrep = q_nm[:, it:it + 1, :]
                    qrep_b, qbc3_b = broadcast_tensor_aps(qrep, qbc3)
                    nc.vector.tensor_tensor(ot3, qrep_b, qbc3_b, op=OP.add)
                else:
                    for s in range(C):
                        col = oc * JCH + s * 512
                        tag = pw_tags[s]
                        p = psum.tile([128, 512], f32, tag=tag,
                                      bufs=pw_bufs[tag],
                                      name=f"pep{oc}_{it}_{s}")
                        nc.tensor.matmul(p[:], lhsT=lhsTq[:, ts(it, 128)],
                                         rhs=patt[:, col:col + 512],
                                         start=True, stop=True)
                        nc.scalar.copy(ot[:, ts(s, 512)], p[:])
                big = nc.sync.dma_start(
                    out=out2[ts(it, 128), oc * JCH:(oc + 1) * JCH], in_=ot[:])
                slab_dmas[it].append(big)

        # diagonal -1 rows: data-driven indirect scatter after slab writes
        for it in range(WPC):
            ind = nc.gpsimd.indirect_dma_start(
                out=out_flat, out_offset=IndirectOffsetOnAxis(
                    ap=sb_diag[:, it:it + 1], axis=0),
                in_=neg1[:], in_offset=None)
            for b in slab_dmas[it]:
                add_dep(ind.ins, b.ins, reason="diag fixup after slab write")

    nc.compile()
    return nc


# ----------------------------------------------------------------------------
# entry point
# ----------------------------------------------------------------------------
def kernel(**inputs):
    from concourse import bass_utils

    g = {k: np.asarray(v) for k, v in inputs.items()}
    cores, T_w = _prep(g["edge_index"], g["edge_attr"], g)
    wts = _prep_weights(g)

    if T_w not in _cache:
        _cache[T_w] = _build(T_w)
    nc = _cache[T_w]

    in_maps = []
    for r in range(NCORES):
        m = dict(wts)
        m.update(cores[r])
        in_maps.append(m)

    res = bass_utils.run_bass_kernel_spmd(nc, in_maps,
                                          core_ids=list(range(NCORES)))
    kernel._last_results = res
    out = np.concatenate([res.results[r]["out"] for r in range(NCORES)],
                         axis=0)
    return out.reshape(N * N, C).astype(np.float32)


kernel._last_results = None


# revision 38
# speedup vs baseline: 1.1541x; 1.1541x over previous
"""Trainium2 Bass kernel for nn_GAT_WLN (GNN message passing, 8 NeuronCores).

Strategy (graph/data parallel per the sharding hint):
  - Nodes sharded 512/core; edges sharded by destination node into 128-node
    windows; one edge stream (real edges only, window-padded) shared by both
    message-passing phases. GAT self-loops are handled by dedicated per-
    window tiles that read the core's local node data — no gather, and they
    run inside the AllGather hole.
  - Input-linear edge/node encodings precomputed on host (same category as
    the baseline's h0/P precompute): Z = P[src] + ea@W1b.T + b1 (relu'd on
    device), SPg = ea@W2c.T + b2c. Z/SP stream through small rotating SBUF
    tiles. This removes all phase-B indirect gathers.
  - Per-window drains emit h1 node-major in one matmul chain (bias via a
    ones-row matmul), then R|g|a_s(hi/lo split, near-f32 exact) are shipped
    bf16 through one AllGather ([N, 516] table, 4x smaller than fp32 R|g).
  - Phase C gathers [128, 516] bf16 rows per edge tile (SWDGE); attention
    softmax without max-subtraction (validated |e| small).
  - q is allgathered (tiny, bf16).
  - Pairwise map q[x]+q[y]: 2 column-chunks on the PE via interleave
    matmuls vs a static eye pattern (drained on ACT), 6 chunks via one-wide
    DVE broadcast adds vs a PSUM-broadcast q row. Output written bf16 (host
    converts to f32). Diagonal -1 rows via data-driven indirect scatter.
  - PE HAM clock gate tripped to 2.4 GHz with junk matmul blocks at start
    and again before the pairwise phase (it cools during AG3).
"""
import numpy as np
import ml_dtypes

N, E = 4096, 32768
F, D, H, C = 82, 6, 256, 5
SLOPE = 0.2
NCORES = 8
NPC = N // NCORES          # 512 nodes per core
WIN = 128                  # dst window
WPC = NPC // WIN           # 4 windows per core
AG2W = 516                 # allgathered node payload width (bf16)

BF16 = ml_dtypes.bfloat16

_cache = {}


# ----------------------------------------------------------------------------
# host-side preprocessing
# ----------------------------------------------------------------------------
def _prep(edge_index, edge_attr, g):
    src = np.asarray(edge_index[0], dtype=np.int64)
    dst = np.asarray(edge_index[1], dtype=np.int64)
    ea = np.asarray(edge_attr, dtype=np.float32)

    order = np.argsort(dst, kind="stable")
    srcs, dsts = src[order], dst[order]
    eas = ea[order]

    groups = [[None] * WPC for _ in range(NCORES)]
    gidx = dsts // WIN
    bounds = np.searchsorted(gidx, np.arange(NCORES * WPC + 1))
    cnt = np.zeros((NCORES, WPC), np.int64)
    for r in range(NCORES):
        for w in range(WPC):
            lo, hi = bounds[r * WPC + w], bounds[r * WPC + w + 1]
            groups[r][w] = (lo, hi)
            cnt[r, w] = hi - lo

    T_w = int(-(-cnt.max() // 128))              # edge tiles per window
    T_tot = WPC * T_w

    # host input encodings (input-linear, same category as h0/P)
    f32 = np.float32
    x = np.asarray(g["x"], f32)
    h0f = np.maximum(x @ np.asarray(g["W_lin"], f32).T, 0.0)
    W1a = np.asarray(g["wl1_W1"], f32)[:, :H]
    W1b = np.asarray(g["wl1_W1"], f32)[:, H:]
    P_f32 = h0f @ W1a.T                                     # [N, H]
    qe_all = eas @ W1b.T + np.asarray(g["wl1_b1"], f32)     # [E, H]
    Zrows = (P_f32[srcs] + qe_all).astype(BF16)             # [E, H]
    sp_all = (eas @ np.asarray(g["wl2_W2"], f32).T
              + np.asarray(g["wl2_b2"], f32)).astype(BF16)  # [E, H]

    cores = []
    for r in range(NCORES):
        Z_sb = np.zeros((128, T_tot * H), BF16)
        SP_sb = np.zeros((128, T_tot * H), BF16)
        src_sb = np.zeros((128, T_tot), np.int32)
        ohBC = np.zeros((128, T_tot * 128), np.float32)
        ohGT = np.zeros((128, T_tot * 128), np.float32)
        Z3 = Z_sb.reshape(128, T_tot, H)
        SP3 = SP_sb.reshape(128, T_tot, H)
        for w in range(WPC):
            lo, hi = groups[r][w]
            nloc = (dsts[lo:hi] % WIN).astype(np.int64)
            pos = w * T_w * 128 + np.arange(hi - lo)
            tt, cc = pos // 128, pos % 128
            Z3[cc, tt] = Zrows[lo:hi]
            SP3[cc, tt] = sp_all[lo:hi]
            src_sb[cc, tt] = srcs[lo:hi]
            ohBC[cc, tt * 128 + nloc] = 1.0
            ohGT[nloc, tt * 128 + cc] = 1.0
        iloc = np.arange(NPC)
        diag_sb = ((iloc * N) + (r * NPC + iloc)).astype(np.int32) \
            .reshape(WPC, 128).T
        cores.append(dict(
            diag_sb=np.ascontiguousarray(diag_sb),
            Z_sb=Z_sb,
            SP_sb=SP_sb,
            src_sb=src_sb,
            ohBC=ohBC.astype(BF16),
            ohGATT=ohGT.astype(BF16),
            h0Tl=np.ascontiguousarray(
                h0f[r * NPC:(r + 1) * NPC].T.reshape(2, 128, NPC)
                .transpose(1, 0, 2).astype(BF16)),
        ))
    return cores, T_w


def _prep_weights(g):
    f32 = np.float32

    def kchunks(wT, nk, extra=None):
        # wT: [K, M] -> [128, nk, M(+1)] chunked along K; extra: [K] column
        K, M = wT.shape
        assert K == nk * 128
        w = np.asarray(wT, f32)
        if extra is not None:
            w = np.concatenate([w, np.asarray(extra, f32)[:, None]], axis=1)
        return np.ascontiguousarray(
            w.reshape(nk, 128, -1).transpose(1, 0, 2).astype(BF16))

    gat_W = np.asarray(g["gat_W"], f32)
    v_as = gat_W.T @ np.asarray(g["gat_asrc"], f32)   # [H]: a_s = h1 @ v_as
    v_ad = gat_W.T @ np.asarray(g["gat_adst"], f32)   # [H]: a_d = h1 @ v_ad

    out = {}
    out["w2T"] = kchunks(np.asarray(g["wl1_W2"], f32).T, 4)      # [128,4,256]
    out["b2row"] = np.asarray(g["wl1_b2"], f32)[None, :].astype(BF16)
    out["w3v"] = kchunks(np.asarray(g["wl2_W3"], f32).T, 2, v_as)  # [128,2,257]
    b3r = np.zeros((1, H + 1), f32)
    b3r[0, :H] = np.asarray(g["wl2_b3"], f32)
    out["b3row"] = b3r.astype(BF16)                               # [1,257]
    out["gatwv"] = kchunks(gat_W.T, 2, v_ad)                      # [128,2,257]
    out["wl2T"] = kchunks(np.asarray(g["W_lin2"], f32).T, 2)
    out["wl3T"] = kchunks(np.asarray(g["W_lin3"], f32).T, 2)
    out["b3c"] = np.ascontiguousarray(
        np.asarray(g["wl2_b3"], f32).reshape(2, 128).T)
    out["qconstc"] = np.ascontiguousarray(
        (((np.asarray(g["gat_b"], f32) @ np.asarray(g["W_lin2"], f32).T)
          @ np.asarray(g["W_lin3"], f32).T)[:, None]).astype(f32))
    return out


# ----------------------------------------------------------------------------
# device program
# ----------------------------------------------------------------------------
def _build(T_w):
    import concourse.bass as bass
    import concourse.tile as tile
    from concourse import bacc, mybir
    from concourse.bass import IndirectOffsetOnAxis, ts, broadcast_tensor_aps
    from concourse.bass import _add_dep_helper as add_dep
    from concourse.masks import make_identity
    from contextlib import ExitStack

    f32 = mybir.dt.float32
    bf16 = mybir.dt.bfloat16
    i32 = mybir.dt.int32
    AF = mybir.ActivationFunctionType
    OP = mybir.AluOpType

    T_tot = WPC * T_w
    JCH = 512 * C          # 2560 output cols per chunk
    NJC = N // 512         # 8 chunks per row-tile

    nc = bacc.Bacc("TRN2", target_bir_lowering=False, debug=False,
                   enable_asserts=False, num_devices=NCORES)

    def inp(name, shape, dt=bf16):
        return nc.dram_tensor(name, list(shape), dt, kind="ExternalInput").ap()

    d_Z = inp("Z_sb", [128, T_tot * H])
    d_SP = inp("SP_sb", [128, T_tot * H])
    d_src = inp("src_sb", [128, T_tot], i32)
    d_ohBC = inp("ohBC", [128, T_tot * 128])
    d_ohGT = inp("ohGATT", [128, T_tot * 128])
    d_h0Tl = inp("h0Tl", [128, 2, NPC])
    d_w2T = inp("w2T", [128, 4, H])
    d_b2row = inp("b2row", [1, H])
    d_w3v = inp("w3v", [128, 2, H + 1])
    d_b3row = inp("b3row", [1, H + 1])
    d_gatwv = inp("gatwv", [128, 2, H + 1])
    d_wl2T = inp("wl2T", [128, 2, H])
    d_wl3T = inp("wl3T", [128, 2, C])
    d_b3c = inp("b3c", [128, 2], f32)
    d_qconstc = inp("qconstc", [C, 1], f32)
    d_diag = inp("diag_sb", [128, WPC], i32)

    out_h = nc.dram_tensor("out", [NPC * N, C], bf16, kind="ExternalOutput")
    out_flat = out_h.ap()
    out2 = out_flat.rearrange("(i j) c -> i (j c)", i=NPC)

    with tile.TileContext(nc) as tc, ExitStack() as ctx:
        const = ctx.enter_context(tc.tile_pool(name="const", bufs=1))
        nodes = ctx.enter_context(tc.tile_pool(name="nodes", bufs=1))
        epool = ctx.enter_context(tc.tile_pool(name="edge", bufs=3))
        pwpool = ctx.enter_context(tc.tile_pool(name="pw", bufs=1))
        psum = ctx.enter_context(tc.tile_pool(name="psum", bufs=1, space="PSUM"))
        dram = ctx.enter_context(tc.tile_pool(name="dram", bufs=1, space="DRAM"))

        _n = [0]

        def pt(shape, tag="mm", dt=f32, bufs=4):
            _n[0] += 1
            return psum.tile(list(shape), dt, tag=tag, bufs=bufs,
                             name=f"ps{_n[0]}")

        def cload(name, ap, dt=bf16):
            t = const.tile(list(ap.shape), dt, name=name)
            nc.sync.dma_start(out=t[:], in_=ap)
            return t

        # collective buffers
        ag2_in = dram.tile([NPC, AG2W], bf16)
        ag2_out = dram.tile([N, AG2W], bf16, addr_space="Shared")
        ag3_in = dram.tile([NPC, C], bf16)
        ag3_out = dram.tile([N, C], bf16, addr_space="Shared")
        RG = [list(range(NCORES))]

        # phase-B-critical loads only; everything phase C needs is queued
        # after the phase B loop so its Z stream isn't stuck behind them
        NCH = 4
        sb_ohBC = const.tile([128, T_tot * 128], bf16, name="sb_ohBC")
        bc = (T_tot * 128) // NCH
        for ch in range(NCH):
            nc.sync.dma_start(out=sb_ohBC[:, ch * bc:(ch + 1) * bc],
                              in_=d_ohBC[:, ch * bc:(ch + 1) * bc])
        sb_w2T = cload("sb_w2T", d_w2T)
        sb_b2row = cload("sb_b2row", d_b2row)
        h0Tl = cload("h0Tl", d_h0Tl)
        sb_w3v = cload("sb_w3v", d_w3v)
        sb_b3row = cload("sb_b3row", d_b3row)
        sb_gatwv = cload("sb_gatwv", d_gatwv)
        identity = const.tile([128, 128], bf16)
        make_identity(nc, identity[:])
        identity_f = const.tile([128, 128], f32)
        make_identity(nc, identity_f[:])
        ones1 = const.tile([1, 128], bf16)
        nc.vector.memset(ones1[:], 1.0)

        # PE warm-up: ~6us of back-to-back matmuls trips the HAM clock gate
        # to 2.4 GHz before real work arrives (PE is otherwise idle here)
        warm_rhs = const.tile([128, 512], bf16)
        nc.vector.memset(warm_rhs[:], 0.0)
        for _ in range(10):
            p = pt([128, 512], tag="aggG", bufs=2)
            nc.tensor.matmul(p[:], lhsT=identity[:], rhs=warm_rhs[:],
                             start=True, stop=True)

        def transpose_128(dst_ap, src_ap):
            p = pt([src_ap.shape[1], src_ap.shape[0]], dt=bf16)
            nc.tensor.transpose(p[:], src_ap,
                                identity[:src_ap.shape[0], :src_ap.shape[0]])
            nc.vector.tensor_copy(dst_ap, p[:])

        # ========== phase B: relu(Z) -> agg -> h1 -> R|g|a_s per window =====
        agg_nm = nodes.tile([128, WPC, H], bf16)
        aggT = nodes.tile([128, 2, NPC], bf16)
        h1_nm = nodes.tile([128, WPC, H], bf16)
        h1T = nodes.tile([128, 2, NPC], bf16)
        ag2row = nodes.tile([128, WPC, 515], bf16)
        ad_bf = nodes.tile([128, WPC], bf16)
        ad_f32 = nodes.tile([128, WPC], f32)
        nc.vector.memset(ag2row[:, :, 512:513], 1.0)
        aggp = [None] * WPC
        for t in range(T_tot):
            w = t // T_w
            if t % T_w == 0:
                aggp[w] = pt([128, H], tag="agg", bufs=2)
            zt = epool.tile([128, H], bf16, tag="zin", bufs=8)
            nc.sync.dma_start(out=zt[:], in_=d_Z[:, ts(t, H)])
            msg = epool.tile([128, H], bf16, tag="msg")
            nc.scalar.activation(msg[:], zt[:], AF.Relu)
            nc.tensor.matmul(aggp[w][:], lhsT=sb_ohBC[:, ts(t, 128)],
                             rhs=msg[:],
                             start=(t % T_w == 0), stop=(t % T_w == T_w - 1),
                             skip_group_check=True)
            if t % T_w != T_w - 1:
                continue
            # ---- window w drained: h1 -> R|g|a_s -> AG2 input rows ----
            wsl = ts(w, 128)
            nc.scalar.copy(agg_nm[:, w, :], aggp[w][:])
            for m in range(2):
                transpose_128(aggT[:, m, wsl], agg_nm[:, w, ts(m, 128)])
            ph = pt([128, H])
            for kc in range(4):
                lhs = aggT[:, kc, wsl] if kc < 2 else h0Tl[:, kc - 2, wsl]
                nc.tensor.matmul(ph[:], lhsT=lhs, rhs=sb_w2T[:, kc, :],
                                 start=(kc == 0), stop=False)
            nc.tensor.matmul(ph[:], lhsT=ones1[:], rhs=sb_b2row[:],
                             start=False, stop=True)
            nc.scalar.activation(h1_nm[:, w, :], ph[:], AF.Relu)
            for m in range(2):
                transpose_128(h1T[:, m, wsl], h1_nm[:, w, ts(m, 128)])
            pr = pt([128, H + 1], tag="agg", bufs=2)
            for kc in range(2):
                nc.tensor.matmul(pr[:], lhsT=h1T[:, kc, wsl],
                                 rhs=sb_w3v[:, kc, :],
                                 start=(kc == 0), stop=False)
            nc.tensor.matmul(pr[:], lhsT=ones1[:], rhs=sb_b3row[:],
                             start=False, stop=True)
            nc.scalar.copy(ag2row[:, w, 0:H], pr[:, 0:H])
            nc.vector.tensor_copy(ag2row[:, w, 513:514], pr[:, H:H + 1])
            nc.vector.tensor_tensor(ag2row[:, w, 514:515], pr[:, H:H + 1],
                                    ag2row[:, w, 513:514], op=OP.subtract)
            pg = pt([128, H + 1], tag="agg", bufs=2)
            for kc in range(2):
                nc.tensor.matmul(pg[:], lhsT=h1T[:, kc, wsl],
                                 rhs=sb_gatwv[:, kc, :],
                                 start=(kc == 0), stop=(kc == 1))
            nc.scalar.copy(ag2row[:, w, H:2 * H], pg[:, 0:H])
            nc.vector.tensor_copy(ad_bf[:, w:w + 1], pg[:, H:H + 1])
            nc.vector.tensor_copy(ad_f32[:, w:w + 1], pg[:, H:H + 1])
            nc.sync.dma_start(out=ag2_in[wsl, 0:515], in_=ag2row[:, w, :])

        nc.gpsimd.collective_compute("AllGather", OP.bypass, replica_groups=RG,
                                     ins=[ag2_in.opt()], outs=[ag2_out.opt()])

        # phase-C / tail const loads (queued behind phase B's Z stream)
        sb_src = cload("sb_src", d_src, i32)
        sb_ohGT = cload("sb_ohGT", d_ohGT)
        sb_wl2T = cload("sb_wl2T", d_wl2T)
        sb_wl3T = cload("sb_wl3T", d_wl3T)
        sb_b3c = cload("sb_b3c", d_b3c, f32)
        sb_qconst = cload("sb_qconst", d_qconstc, f32)
        sb_diag = cload("sb_diag", d_diag, i32)
        neg1 = const.tile([128, C], bf16)
        nc.vector.memset(neg1[:], -1.0)

        # ========== self-loop GAT tiles: local data only, run in the
        # collective hole (no gather, no WL-output contribution) ==========
        aggS_g = nodes.tile([128, WPC, H + 1], f32)
        for w in range(WPC):
            tas = epool.tile([128, 1], f32, tag="tas")
            nc.vector.scalar_tensor_tensor(tas[:], in0=ag2row[:, w, 513:514],
                                           scalar=1.0,
                                           in1=ag2row[:, w, 514:515],
                                           op0=OP.mult, op1=OP.add)
            eatt = epool.tile([128, 1], f32, tag="eatt")
            nc.scalar.activation(eatt[:], tas[:], AF.Identity,
                                 bias=ad_f32[:, w:w + 1])
            el = epool.tile([128, 1], f32, tag="el")
            nc.vector.scalar_tensor_tensor(el[:], in0=eatt[:], scalar=SLOPE,
                                           in1=eatt[:], op0=OP.mult,
                                           op1=OP.max)
            ex = epool.tile([128, 1], f32, tag="ex")
            nc.scalar.activation(ex[:], el[:], AF.Exp)
            # fold the exp scaling into the one-hot lhsT instead of scaling
            # the [128,257] message; rhs [g|1] yields numerator+denominator
            ohx = epool.tile([128, 128], bf16, tag="ohx", bufs=4)
            nc.scalar.activation(ohx[:], identity[:], AF.Copy, scale=ex[:])
            ps = pt([128, H + 1], tag="aggG", bufs=2)
            nc.tensor.matmul(ps[:], lhsT=ohx[:], rhs=ag2row[:, w, H:2 * H + 1],
                             start=True, stop=True)
            nc.scalar.copy(aggS_g[:, w, :], ps[:])

        # a_d per edge — no AG2 dependency, also fills the collective hole
        ad_e_all = nodes.tile([128, T_tot], f32)
        for t in range(T_tot):
            w = t // T_w
            pd = pt([128, 1])
            nc.tensor.matmul(pd[:], lhsT=sb_ohGT[:, ts(t, 128)],
                             rhs=ad_bf[:, w:w + 1], start=True, stop=True)
            nc.vector.tensor_copy(ad_e_all[:, t:t + 1], pd[:])

        # ========== phase C gathered edges ================================
        u_nm = nodes.tile([128, WPC, H], bf16, tag="nmA2")
        glob_nm = nodes.tile([128, WPC, H], bf16, tag="nmB2")
        uT = nodes.tile([128, 2, NPC], bf16, tag="ftA")
        globT = nodes.tile([128, 2, NPC], bf16, tag="ftB")
        preT = nodes.tile([128, 2, NPC], bf16)
        t1T = nodes.tile([128, 2, NPC], bf16)
        qsb = nodes.tile([C, NPC], f32)
        q_nm = nodes.tile([128, WPC, C], bf16)
        aggcp = [None] * WPC
        agggp = [None] * WPC
        for t in range(T_tot):
            w = t // T_w
            k = t % T_w
            if k == 0:
                aggcp[w] = pt([128, H], tag="agg", bufs=2)
                agggp[w] = pt([128, H + 1], tag="aggG", bufs=2)
            gR = epool.tile([128, AG2W], bf16, tag="gath2", bufs=8)
            nc.gpsimd.indirect_dma_start(
                out=gR[:], out_offset=None, in_=ag2_out[:, :],
                in_offset=IndirectOffsetOnAxis(ap=sb_src[:, t:t + 1], axis=0))
            spt = epool.tile([128, H], bf16, tag="spin", bufs=8)
            nc.sync.dma_start(out=spt[:], in_=d_SP[:, ts(t, H)])
            msg2 = epool.tile([128, H], bf16, tag="msg")
            nc.vector.tensor_tensor(msg2[:], gR[:, 0:H], spt[:], op=OP.mult)
            nc.tensor.matmul(aggcp[w][:], lhsT=sb_ohBC[:, ts(t, 128)],
                             rhs=msg2[:],
                             start=(k == 0), stop=(k == T_w - 1),
                             skip_group_check=True)
            tas = epool.tile([128, 1], f32, tag="tas")
            nc.vector.scalar_tensor_tensor(tas[:], in0=gR[:, 513:514],
                                           scalar=1.0, in1=gR[:, 514:515],
                                           op0=OP.mult, op1=OP.add)
            eatt = epool.tile([128, 1], f32, tag="eatt")
            nc.scalar.activation(eatt[:], tas[:], AF.Identity,
                                 bias=ad_e_all[:, t:t + 1])
            el = epool.tile([128, 1], f32, tag="el")
            nc.vector.scalar_tensor_tensor(el[:], in0=eatt[:], scalar=SLOPE,
                                           in1=eatt[:], op0=OP.mult,
                                           op1=OP.max)
            ex = epool.tile([128, 1], f32, tag="ex")
            nc.scalar.activation(ex[:], el[:], AF.Exp)
            ohx = epool.tile([128, 128], bf16, tag="ohx", bufs=4)
            nc.scalar.activation(ohx[:], sb_ohBC[:, ts(t, 128)], AF.Copy,
                                 scale=ex[:])
            nc.tensor.matmul(agggp[w][:], lhsT=ohx[:],
                             rhs=gR[:, H:2 * H + 1],
                             start=(k == 0), stop=(k == T_w - 1),
                             skip_group_check=True)
            if k != T_w - 1:
                continue
            # ---- window complete: combine with self partials ----
            nc.vector.tensor_mul(u_nm[:, w, :], aggcp[w][:], h1_nm[:, w, :])
            tmpg = epool.tile([128, H + 1], f32, tag="tmpg", bufs=2)
            nc.vector.tensor_add(tmpg[:], agggp[w][:], aggS_g[:, w, :])
            rec = epool.tile([128, 1], f32, tag="rec")
            nc.vector.reciprocal(rec[:], tmpg[:, H:H + 1])
            nc.vector.tensor_scalar(glob_nm[:, w, :], tmpg[:, 0:H],
                                    rec[:], None, op0=OP.mult)

        # ========== tail: q (per-window slices, emitted post-loop so the
        # scheduler runs w0-2 during remaining phase-C gathers) ==========
        for w in range(WPC):
            wsl = ts(w, 128)
            for m in range(2):
                transpose_128(uT[:, m, wsl], u_nm[:, w, ts(m, 128)])
                transpose_128(globT[:, m, wsl], glob_nm[:, w, ts(m, 128)])
            for m in range(2):
                p = pt([128, 128])
                for kc in range(2):
                    nc.tensor.matmul(p[:], lhsT=sb_w3v[:, kc, ts(m, 128)],
                                     rhs=uT[:, kc, wsl],
                                     start=(kc == 0), stop=(kc == 1))
                lt = epool.tile([128, 128], bf16, tag="loc", bufs=2)
                nc.scalar.activation(lt[:], p[:], AF.Identity,
                                     bias=sb_b3c[:, m:m + 1])
                nc.vector.tensor_add(preT[:, m, wsl], lt[:], globT[:, m, wsl])
            for m in range(2):
                p = pt([128, 128])
                for kc in range(2):
                    nc.tensor.matmul(p[:], lhsT=sb_wl2T[:, kc, ts(m, 128)],
                                     rhs=preT[:, kc, wsl],
                                     start=(kc == 0), stop=(kc == 1))
                nc.scalar.copy(t1T[:, m, wsl], p[:])
            qp5 = pt([C, 128])
            for kc in range(2):
                nc.tensor.matmul(qp5[:], lhsT=sb_wl3T[:, kc, :],
                                 rhs=t1T[:, kc, wsl],
                                 start=(kc == 0), stop=(kc == 1))
            nc.vector.tensor_scalar(qsb[:, wsl], qp5[:], sb_qconst[:], None,
                                    op0=OP.add)
            pq = pt([128, C])
            nc.tensor.transpose(pq[:], qsb[:, wsl], identity_f[:C, :C])
            nc.vector.tensor_copy(q_nm[:, w, :], pq[:])
            nc.sync.dma_start(out=ag3_in[wsl, :], in_=q_nm[:, w, :])

        nc.gpsimd.collective_compute("AllGather", OP.bypass, replica_groups=RG,
                                     ins=[ag3_in.opt()], outs=[ag3_out.opt()])

        # ========== pairwise map =====
        # patt: q[j,c] flattened on partition 0 (bcast-matmul rhs row)
        patt = nodes.tile([1, C * N], bf16, tag="bigbuf")
        ag3o_flat = ag3_out[:, :].rearrange("n c -> (n c)")[None, :]
        nc.sync.dma_start(out=patt[0:1, :], in_=ag3o_flat)
        patt5 = patt[0:1, :]

        pw_tags = ["mm", "agg", "aggG", "mm", "agg"]
        pw_bufs = {"mm": 4, "agg": 2, "aggG": 2}
        slab_dmas = [[] for _ in range(WPC)]
        for oc in range(NJC):
            qbc = pwpool.tile([128, JCH], bf16, tag="qbc", bufs=2,
                              name=f"qbc{oc}")
            for s in range(C):
                tag = pw_tags[s]
                p = psum.tile([128, 512], f32, tag=tag, bufs=pw_bufs[tag],
                              name=f"pwp{oc}_{s}")
                nc.tensor.matmul(p[:], lhsT=ones1[:],
                                 rhs=patt5[:, oc * JCH + s * 512:
                                           oc * JCH + (s + 1) * 512],
                                 start=True, stop=True)
                if s >= 3:
                    nc.vector.tensor_copy(qbc[:, ts(s, 512)], p[:])
                else:
                    nc.scalar.copy(qbc[:, ts(s, 512)], p[:])
            qbc3 = qbc[:].rearrange("p (j c) -> p j c", c=C)
            for it in range(WPC):
                ot = pwpool.tile([128, JCH], bf16, tag="ot", bufs=6,
                                 name=f"ot{oc}_{it}")
                ot3 = ot[:].rearrange("p (j c) -> p j c", c=C)
                if (oc * WPC + it) % 4 == 3:
                    # ACT path: q[i,c] enters as a per-partition bias, one
                    # strided [128,512] op per c
                    for cc in range(C):
                        nc.scalar.activation(ot3[:, :, cc], qbc3[:, :, cc],
                                             AF.Identity,
                                             bias=q_nm[:, it, cc:cc + 1])
                else:
                    qrep = q_nm[:, it:it + 1, :]
                    qrep_b, qbc3_b = broadcast_tensor_aps(qrep, qbc3)
                    nc.vector.tensor_tensor(ot3, qrep_b, qbc3_b, op=OP.add)
                big = nc.sync.dma_start(
                    out=out2[ts(it, 128), oc * JCH:(oc + 1) * JCH], in_=ot[:])
                slab_dmas[it].append(big)

        # diagonal -1 rows: data-driven indirect scatter after slab writes
        for it in range(WPC):
            ind = nc.gpsimd.indirect_dma_start(
                out=out_flat, out_offset=IndirectOffsetOnAxis(
                    ap=sb_diag[:, it:it + 1], axis=0),
                in_=neg1[:], in_offset=None)
            for b in slab_dmas[it]:
                add_dep(ind.ins, b.ins, reason="diag fixup after slab write")

    nc.compile()
    return nc


# ----------------------------------------------------------------------------
# entry point
# ----------------------------------------------------------------------------
def kernel(**inputs):
    from concourse import bass_utils

    g = {k: np.asarray(v) for k, v in inputs.items()}
    cores, T_w = _prep(g["edge_index"], g["edge_attr"], g)
    wts = _prep_weights(g)

    if T_w not in _cache:
        _cache[T_w] = _build(T_w)
    nc = _cache[T_w]

    in_maps = []
    for r in range(NCORES):
        m = dict(wts)
        m.update(cores[r])
        in_maps.append(m)

    res = bass_utils.run_bass_kernel_spmd(nc, in_maps,
                                          core_ids=list(range(NCORES)))
    kernel._last_results = res
    out = np.concatenate([res.results[r]["out"] for r in range(NCORES)],
                         axis=0)
    return out.reshape(N * N, C).astype(np.float32)


kernel._last_results = None


# revision 39
# speedup vs baseline: 1.1635x; 1.0082x over previous
"""Trainium2 Bass kernel for nn_GAT_WLN (GNN message passing, 8 NeuronCores).

Strategy (graph/data parallel per the sharding hint):
  - Nodes sharded 512/core; edges sharded by destination node into 128-node
    windows; one edge stream (real edges only, window-padded) shared by both
    message-passing phases. GAT self-loops are handled by dedicated per-
    window tiles that read the core's local node data — no gather, and they
    run inside the AllGather hole.
  - Input-linear edge/node encodings precomputed on host (same category as
    the baseline's h0/P precompute): Z = P[src] + ea@W1b.T + b1 (relu'd on
    device), SPg = ea@W2c.T + b2c. Z/SP stream through small rotating SBUF
    tiles. This removes all phase-B indirect gathers.
  - Per-window drains emit h1 node-major in one matmul chain (bias via a
    ones-row matmul), then R|g|a_s(hi/lo split, near-f32 exact) are shipped
    bf16 through one AllGather ([N, 516] table, 4x smaller than fp32 R|g).
  - Phase C gathers [128, 516] bf16 rows per edge tile (SWDGE); attention
    softmax without max-subtraction (validated |e| small).
  - q is allgathered (tiny, bf16).
  - Pairwise map q[x]+q[y]: 2 column-chunks on the PE via interleave
    matmuls vs a static eye pattern (drained on ACT), 6 chunks via one-wide
    DVE broadcast adds vs a PSUM-broadcast q row. Output written bf16 (host
    converts to f32). Diagonal -1 rows via data-driven indirect scatter.
  - PE HAM clock gate tripped to 2.4 GHz with junk matmul blocks at start
    and again before the pairwise phase (it cools during AG3).
"""
import numpy as np
import ml_dtypes

N, E = 4096, 32768
F, D, H, C = 82, 6, 256, 5
SLOPE = 0.2
NCORES = 8
NPC = N // NCORES          # 512 nodes per core
WIN = 128                  # dst window
WPC = NPC // WIN           # 4 windows per core
AG2W = 516                 # allgathered node payload width (bf16)

BF16 = ml_dtypes.bfloat16

_cache = {}


# ----------------------------------------------------------------------------
# host-side preprocessing
# ----------------------------------------------------------------------------
def _prep(edge_index, edge_attr, g):
    src = np.asarray(edge_index[0], dtype=np.int64)
    dst = np.asarray(edge_index[1], dtype=np.int64)
    ea = np.asarray(edge_attr, dtype=np.float32)

    order = np.argsort(dst, kind="stable")
    srcs, dsts = src[order], dst[order]
    eas = ea[order]

    groups = [[None] * WPC for _ in range(NCORES)]
    gidx = dsts // WIN
    bounds = np.searchsorted(gidx, np.arange(NCORES * WPC + 1))
    cnt = np.zeros((NCORES, WPC), np.int64)
    for r in range(NCORES):
        for w in range(WPC):
            lo, hi = bounds[r * WPC + w], bounds[r * WPC + w + 1]
            groups[r][w] = (lo, hi)
            cnt[r, w] = hi - lo

    T_w = int(-(-cnt.max() // 128))              # edge tiles per window
    T_tot = WPC * T_w

    # host input encodings (input-linear, same category as h0/P)
    f32 = np.float32
    x = np.asarray(g["x"], f32)
    h0f = np.maximum(x @ np.asarray(g["W_lin"], f32).T, 0.0)
    W1a = np.asarray(g["wl1_W1"], f32)[:, :H]
    W1b = np.asarray(g["wl1_W1"], f32)[:, H:]
    P_f32 = h0f @ W1a.T                                     # [N, H]
    qe_all = eas @ W1b.T + np.asarray(g["wl1_b1"], f32)     # [E, H]
    Zrows = (P_f32[srcs] + qe_all).astype(BF16)             # [E, H]
    sp_all = (eas @ np.asarray(g["wl2_W2"], f32).T
              + np.asarray(g["wl2_b2"], f32)).astype(BF16)  # [E, H]

    cores = []
    for r in range(NCORES):
        Z_sb = np.zeros((128, T_tot * H), BF16)
        SP_sb = np.zeros((128, T_tot * H), BF16)
        src_sb = np.zeros((128, T_tot), np.int32)
        ohBC = np.zeros((128, T_tot * 128), np.float32)
        ohGT = np.zeros((128, T_tot * 128), np.float32)
        Z3 = Z_sb.reshape(128, T_tot, H)
        SP3 = SP_sb.reshape(128, T_tot, H)
        for w in range(WPC):
            lo, hi = groups[r][w]
            nloc = (dsts[lo:hi] % WIN).astype(np.int64)
            pos = w * T_w * 128 + np.arange(hi - lo)
            tt, cc = pos // 128, pos % 128
            Z3[cc, tt] = Zrows[lo:hi]
            SP3[cc, tt] = sp_all[lo:hi]
            src_sb[cc, tt] = srcs[lo:hi]
            ohBC[cc, tt * 128 + nloc] = 1.0
            ohGT[nloc, tt * 128 + cc] = 1.0
        iloc = np.arange(NPC)
        diag_sb = ((iloc * N) + (r * NPC + iloc)).astype(np.int32) \
            .reshape(WPC, 128).T
        cores.append(dict(
            diag_sb=np.ascontiguousarray(diag_sb),
            Z_sb=Z_sb,
            SP_sb=SP_sb,
            src_sb=src_sb,
            ohBC=ohBC.astype(BF16),
            ohGATT=ohGT.astype(BF16),
            h0Tl=np.ascontiguousarray(
                h0f[r * NPC:(r + 1) * NPC].T.reshape(2, 128, NPC)
                .transpose(1, 0, 2).astype(BF16)),
        ))
    return cores, T_w


def _prep_weights(g):
    f32 = np.float32

    def kchunks(wT, nk, extra=None):
        # wT: [K, M] -> [128, nk, M(+1)] chunked along K; extra: [K] column
        K, M = wT.shape
        assert K == nk * 128
        w = np.asarray(wT, f32)
        if extra is not None:
            w = np.concatenate([w, np.asarray(extra, f32)[:, None]], axis=1)
        return np.ascontiguousarray(
            w.reshape(nk, 128, -1).transpose(1, 0, 2).astype(BF16))

    gat_W = np.asarray(g["gat_W"], f32)
    v_as = gat_W.T @ np.asarray(g["gat_asrc"], f32)   # [H]: a_s = h1 @ v_as
    v_ad = gat_W.T @ np.asarray(g["gat_adst"], f32)   # [H]: a_d = h1 @ v_ad

    out = {}
    out["w2T"] = kchunks(np.asarray(g["wl1_W2"], f32).T, 4)      # [128,4,256]
    out["b2row"] = np.asarray(g["wl1_b2"], f32)[None, :].astype(BF16)
    out["w3v"] = kchunks(np.asarray(g["wl2_W3"], f32).T, 2, v_as)  # [128,2,257]
    b3r = np.zeros((1, H + 1), f32)
    b3r[0, :H] = np.asarray(g["wl2_b3"], f32)
    out["b3row"] = b3r.astype(BF16)                               # [1,257]
    out["gatwv"] = kchunks(gat_W.T, 2, v_ad)                      # [128,2,257]
    out["wl2T"] = kchunks(np.asarray(g["W_lin2"], f32).T, 2)
    out["wl3T"] = kchunks(np.asarray(g["W_lin3"], f32).T, 2)
    out["b3c"] = np.ascontiguousarray(
        np.asarray(g["wl2_b3"], f32).reshape(2, 128).T)
    out["qconstc"] = np.ascontiguousarray(
        (((np.asarray(g["gat_b"], f32) @ np.asarray(g["W_lin2"], f32).T)
          @ np.asarray(g["W_lin3"], f32).T)[:, None]).astype(f32))
    return out


# ----------------------------------------------------------------------------
# device program
# ----------------------------------------------------------------------------
def _build(T_w):
    import concourse.bass as bass
    import concourse.tile as tile
    from concourse import bacc, mybir
    from concourse.bass import IndirectOffsetOnAxis, ts, broadcast_tensor_aps
    from concourse.bass import _add_dep_helper as add_dep
    from concourse.masks import make_identity
    from contextlib import ExitStack

    f32 = mybir.dt.float32
    bf16 = mybir.dt.bfloat16
    i32 = mybir.dt.int32
    AF = mybir.ActivationFunctionType
    OP = mybir.AluOpType

    T_tot = WPC * T_w
    JCH = 512 * C          # 2560 output cols per chunk
    NJC = N // 512         # 8 chunks per row-tile

    nc = bacc.Bacc("TRN2", target_bir_lowering=False, debug=False,
                   enable_asserts=False, num_devices=NCORES)

    def inp(name, shape, dt=bf16):
        return nc.dram_tensor(name, list(shape), dt, kind="ExternalInput").ap()

    d_Z = inp("Z_sb", [128, T_tot * H])
    d_SP = inp("SP_sb", [128, T_tot * H])
    d_src = inp("src_sb", [128, T_tot], i32)
    d_ohBC = inp("ohBC", [128, T_tot * 128])
    d_ohGT = inp("ohGATT", [128, T_tot * 128])
    d_h0Tl = inp("h0Tl", [128, 2, NPC])
    d_w2T = inp("w2T", [128, 4, H])
    d_b2row = inp("b2row", [1, H])
    d_w3v = inp("w3v", [128, 2, H + 1])
    d_b3row = inp("b3row", [1, H + 1])
    d_gatwv = inp("gatwv", [128, 2, H + 1])
    d_wl2T = inp("wl2T", [128, 2, H])
    d_wl3T = inp("wl3T", [128, 2, C])
    d_b3c = inp("b3c", [128, 2], f32)
    d_qconstc = inp("qconstc", [C, 1], f32)
    d_diag = inp("diag_sb", [128, WPC], i32)

    out_h = nc.dram_tensor("out", [NPC * N, C], bf16, kind="ExternalOutput")
    out_flat = out_h.ap()
    out2 = out_flat.rearrange("(i j) c -> i (j c)", i=NPC)

    with tile.TileContext(nc) as tc, ExitStack() as ctx:
        const = ctx.enter_context(tc.tile_pool(name="const", bufs=1))
        nodes = ctx.enter_context(tc.tile_pool(name="nodes", bufs=1))
        epool = ctx.enter_context(tc.tile_pool(name="edge", bufs=3))
        pwpool = ctx.enter_context(tc.tile_pool(name="pw", bufs=1))
        psum = ctx.enter_context(tc.tile_pool(name="psum", bufs=1, space="PSUM"))
        dram = ctx.enter_context(tc.tile_pool(name="dram", bufs=1, space="DRAM"))

        _n = [0]

        def pt(shape, tag="mm", dt=f32, bufs=4):
            _n[0] += 1
            return psum.tile(list(shape), dt, tag=tag, bufs=bufs,
                             name=f"ps{_n[0]}")

        def cload(name, ap, dt=bf16):
            t = const.tile(list(ap.shape), dt, name=name)
            nc.sync.dma_start(out=t[:], in_=ap)
            return t

        # collective buffers
        ag2_in = dram.tile([NPC, AG2W], bf16)
        ag2_out = dram.tile([N, AG2W], bf16, addr_space="Shared")
        ag3_in = dram.tile([NPC, C], bf16)
        ag3_out = dram.tile([N, C], bf16, addr_space="Shared")
        RG = [list(range(NCORES))]

        # phase-B-critical loads only; everything phase C needs is queued
        # after the phase B loop so its Z stream isn't stuck behind them
        NCH = 4
        sb_ohBC = const.tile([128, T_tot * 128], bf16, name="sb_ohBC")
        bc = (T_tot * 128) // NCH
        for ch in range(NCH):
            nc.sync.dma_start(out=sb_ohBC[:, ch * bc:(ch + 1) * bc],
                              in_=d_ohBC[:, ch * bc:(ch + 1) * bc])
        sb_w2T = cload("sb_w2T", d_w2T)
        sb_b2row = cload("sb_b2row", d_b2row)
        h0Tl = cload("h0Tl", d_h0Tl)
        sb_w3v = cload("sb_w3v", d_w3v)
        sb_b3row = cload("sb_b3row", d_b3row)
        sb_gatwv = cload("sb_gatwv", d_gatwv)
        identity = const.tile([128, 128], bf16)
        make_identity(nc, identity[:])
        identity_f = const.tile([128, 128], f32)
        make_identity(nc, identity_f[:])
        ones1 = const.tile([1, 128], bf16)
        nc.vector.memset(ones1[:], 1.0)

        # PE warm-up: ~6us of back-to-back matmuls trips the HAM clock gate
        # to 2.4 GHz before real work arrives (PE is otherwise idle here)
        warm_rhs = const.tile([128, 512], bf16)
        nc.vector.memset(warm_rhs[:], 0.0)
        for _ in range(10):
            p = pt([128, 512], tag="aggG", bufs=2)
            nc.tensor.matmul(p[:], lhsT=identity[:], rhs=warm_rhs[:],
                             start=True, stop=True)

        # phase-C / tail const loads (early: the AG2 transfer otherwise
        # competes with them for HBM bandwidth mid-kernel)
        sb_src = cload("sb_src", d_src, i32)
        sb_ohGT = cload("sb_ohGT", d_ohGT)
        sb_wl2T = cload("sb_wl2T", d_wl2T)
        sb_wl3T = cload("sb_wl3T", d_wl3T)
        sb_b3c = cload("sb_b3c", d_b3c, f32)
        sb_qconst = cload("sb_qconst", d_qconstc, f32)
        sb_diag = cload("sb_diag", d_diag, i32)
        neg1 = const.tile([128, C], bf16)
        nc.vector.memset(neg1[:], -1.0)

        def transpose_128(dst_ap, src_ap):
            p = pt([src_ap.shape[1], src_ap.shape[0]], dt=bf16)
            nc.tensor.transpose(p[:], src_ap,
                                identity[:src_ap.shape[0], :src_ap.shape[0]])
            nc.vector.tensor_copy(dst_ap, p[:])

        # ========== phase B: relu(Z) -> agg -> h1 -> R|g|a_s per window =====
        agg_nm = nodes.tile([128, WPC, H], bf16)
        aggT = nodes.tile([128, 2, NPC], bf16)
        h1_nm = nodes.tile([128, WPC, H], bf16)
        h1T = nodes.tile([128, 2, NPC], bf16)
        ag2row = nodes.tile([128, WPC, 515], bf16)
        ad_bf = nodes.tile([128, WPC], bf16)
        ad_f32 = nodes.tile([128, WPC], f32)
        nc.vector.memset(ag2row[:, :, 512:513], 1.0)
        aggp = [None] * WPC
        for t in range(T_tot):
            w = t // T_w
            if t % T_w == 0:
                aggp[w] = pt([128, H], tag="agg", bufs=2)
            zt = epool.tile([128, H], bf16, tag="zin", bufs=8)
            nc.sync.dma_start(out=zt[:], in_=d_Z[:, ts(t, H)])
            msg = epool.tile([128, H], bf16, tag="msg")
            nc.scalar.activation(msg[:], zt[:], AF.Relu)
            nc.tensor.matmul(aggp[w][:], lhsT=sb_ohBC[:, ts(t, 128)],
                             rhs=msg[:],
                             start=(t % T_w == 0), stop=(t % T_w == T_w - 1),
                             skip_group_check=True)
            if t % T_w != T_w - 1:
                continue
            # ---- window w drained: h1 -> R|g|a_s -> AG2 input rows ----
            wsl = ts(w, 128)
            nc.scalar.copy(agg_nm[:, w, :], aggp[w][:])
            for m in range(2):
                transpose_128(aggT[:, m, wsl], agg_nm[:, w, ts(m, 128)])
            ph = pt([128, H])
            for kc in range(4):
                lhs = aggT[:, kc, wsl] if kc < 2 else h0Tl[:, kc - 2, wsl]
                nc.tensor.matmul(ph[:], lhsT=lhs, rhs=sb_w2T[:, kc, :],
                                 start=(kc == 0), stop=False)
            nc.tensor.matmul(ph[:], lhsT=ones1[:], rhs=sb_b2row[:],
                             start=False, stop=True)
            nc.scalar.activation(h1_nm[:, w, :], ph[:], AF.Relu)
            for m in range(2):
                transpose_128(h1T[:, m, wsl], h1_nm[:, w, ts(m, 128)])
            pr = pt([128, H + 1], tag="agg", bufs=2)
            for kc in range(2):
                nc.tensor.matmul(pr[:], lhsT=h1T[:, kc, wsl],
                                 rhs=sb_w3v[:, kc, :],
                                 start=(kc == 0), stop=False)
            nc.tensor.matmul(pr[:], lhsT=ones1[:], rhs=sb_b3row[:],
                             start=False, stop=True)
            nc.scalar.copy(ag2row[:, w, 0:H], pr[:, 0:H])
            nc.vector.tensor_copy(ag2row[:, w, 513:514], pr[:, H:H + 1])
            nc.vector.tensor_tensor(ag2row[:, w, 514:515], pr[:, H:H + 1],
                                    ag2row[:, w, 513:514], op=OP.subtract)
            pg = pt([128, H + 1], tag="agg", bufs=2)
            for kc in range(2):
                nc.tensor.matmul(pg[:], lhsT=h1T[:, kc, wsl],
                                 rhs=sb_gatwv[:, kc, :],
                                 start=(kc == 0), stop=(kc == 1))
            nc.scalar.copy(ag2row[:, w, H:2 * H], pg[:, 0:H])
            nc.vector.tensor_copy(ad_bf[:, w:w + 1], pg[:, H:H + 1])
            nc.vector.tensor_copy(ad_f32[:, w:w + 1], pg[:, H:H + 1])
            nc.sync.dma_start(out=ag2_in[wsl, 0:515], in_=ag2row[:, w, :])

        nc.gpsimd.collective_compute("AllGather", OP.bypass, replica_groups=RG,
                                     ins=[ag2_in.opt()], outs=[ag2_out.opt()])

        # ========== self-loop GAT tiles: local data only, run in the
        # collective hole (no gather, no WL-output contribution) ==========
        aggS_g = nodes.tile([128, WPC, H + 1], f32)
        for w in range(WPC):
            tas = epool.tile([128, 1], f32, tag="tas")
            nc.vector.scalar_tensor_tensor(tas[:], in0=ag2row[:, w, 513:514],
                                           scalar=1.0,
                                           in1=ag2row[:, w, 514:515],
                                           op0=OP.mult, op1=OP.add)
            eatt = epool.tile([128, 1], f32, tag="eatt")
            nc.scalar.activation(eatt[:], tas[:], AF.Identity,
                                 bias=ad_f32[:, w:w + 1])
            el = epool.tile([128, 1], f32, tag="el")
            nc.vector.scalar_tensor_tensor(el[:], in0=eatt[:], scalar=SLOPE,
                                           in1=eatt[:], op0=OP.mult,
                                           op1=OP.max)
            ex = epool.tile([128, 1], f32, tag="ex")
            nc.scalar.activation(ex[:], el[:], AF.Exp)
            # fold the exp scaling into the one-hot lhsT instead of scaling
            # the [128,257] message; rhs [g|1] yields numerator+denominator
            ohx = epool.tile([128, 128], bf16, tag="ohx", bufs=4)
            nc.scalar.activation(ohx[:], identity[:], AF.Copy, scale=ex[:])
            ps = pt([128, H + 1], tag="aggG", bufs=2)
            nc.tensor.matmul(ps[:], lhsT=ohx[:], rhs=ag2row[:, w, H:2 * H + 1],
                             start=True, stop=True)
            nc.scalar.copy(aggS_g[:, w, :], ps[:])

        # a_d per edge — no AG2 dependency, also fills the collective hole
        ad_e_all = nodes.tile([128, T_tot], f32)
        for t in range(T_tot):
            w = t // T_w
            pd = pt([128, 1])
            nc.tensor.matmul(pd[:], lhsT=sb_ohGT[:, ts(t, 128)],
                             rhs=ad_bf[:, w:w + 1], start=True, stop=True)
            nc.vector.tensor_copy(ad_e_all[:, t:t + 1], pd[:])

        # ========== phase C gathered edges ================================
        u_nm = nodes.tile([128, WPC, H], bf16, tag="nmA2")
        glob_nm = nodes.tile([128, WPC, H], bf16, tag="nmB2")
        uT = nodes.tile([128, 2, NPC], bf16, tag="ftA")
        globT = nodes.tile([128, 2, NPC], bf16, tag="ftB")
        preT = nodes.tile([128, 2, NPC], bf16)
        t1T = nodes.tile([128, 2, NPC], bf16)
        qsb = nodes.tile([C, NPC], f32)
        q_nm = nodes.tile([128, WPC, C], bf16)
        aggcp = [None] * WPC
        agggp = [None] * WPC
        for t in range(T_tot):
            w = t // T_w
            k = t % T_w
            if k == 0:
                aggcp[w] = pt([128, H], tag="agg", bufs=2)
                agggp[w] = pt([128, H + 1], tag="aggG", bufs=2)
            gR = epool.tile([128, AG2W], bf16, tag="gath2", bufs=8)
            nc.gpsimd.indirect_dma_start(
                out=gR[:], out_offset=None, in_=ag2_out[:, :],
                in_offset=IndirectOffsetOnAxis(ap=sb_src[:, t:t + 1], axis=0))
            spt = epool.tile([128, H], bf16, tag="spin", bufs=8)
            nc.sync.dma_start(out=spt[:], in_=d_SP[:, ts(t, H)])
            msg2 = epool.tile([128, H], bf16, tag="msg")
            nc.vector.tensor_tensor(msg2[:], gR[:, 0:H], spt[:], op=OP.mult)
            nc.tensor.matmul(aggcp[w][:], lhsT=sb_ohBC[:, ts(t, 128)],
                             rhs=msg2[:],
                             start=(k == 0), stop=(k == T_w - 1),
                             skip_group_check=True)
            tas = epool.tile([128, 1], f32, tag="tas")
            nc.vector.scalar_tensor_tensor(tas[:], in0=gR[:, 513:514],
                                           scalar=1.0, in1=gR[:, 514:515],
                                           op0=OP.mult, op1=OP.add)
            eatt = epool.tile([128, 1], f32, tag="eatt")
            nc.scalar.activation(eatt[:], tas[:], AF.Identity,
                                 bias=ad_e_all[:, t:t + 1])
            el = epool.tile([128, 1], f32, tag="el")
            nc.vector.scalar_tensor_tensor(el[:], in0=eatt[:], scalar=SLOPE,
                                           in1=eatt[:], op0=OP.mult,
                                           op1=OP.max)
            ex = epool.tile([128, 1], f32, tag="ex")
            nc.scalar.activation(ex[:], el[:], AF.Exp)
            ohx = epool.tile([128, 128], bf16, tag="ohx", bufs=4)
            nc.scalar.activation(ohx[:], sb_ohBC[:, ts(t, 128)], AF.Copy,
                                 scale=ex[:])
            nc.tensor.matmul(agggp[w][:], lhsT=ohx[:],
                             rhs=gR[:, H:2 * H + 1],
                             start=(k == 0), stop=(k == T_w - 1),
                             skip_group_check=True)
            if k != T_w - 1:
                continue
            # ---- window complete: combine with self partials ----
            nc.vector.tensor_mul(u_nm[:, w, :], aggcp[w][:], h1_nm[:, w, :])
            tmpg = epool.tile([128, H + 1], f32, tag="tmpg", bufs=2)
            nc.vector.tensor_add(tmpg[:], agggp[w][:], aggS_g[:, w, :])
            rec = epool.tile([128, 1], f32, tag="rec")
            nc.vector.reciprocal(rec[:], tmpg[:, H:H + 1])
            nc.vector.tensor_scalar(glob_nm[:, w, :], tmpg[:, 0:H],
                                    rec[:], None, op0=OP.mult)

        # ========== tail: q (per-window slices, emitted post-loop so the
        # scheduler runs w0-2 during remaining phase-C gathers) ==========
        for w in range(WPC):
            wsl = ts(w, 128)
            for m in range(2):
                transpose_128(uT[:, m, wsl], u_nm[:, w, ts(m, 128)])
                transpose_128(globT[:, m, wsl], glob_nm[:, w, ts(m, 128)])
            for m in range(2):
                p = pt([128, 128])
                for kc in range(2):
                    nc.tensor.matmul(p[:], lhsT=sb_w3v[:, kc, ts(m, 128)],
                                     rhs=uT[:, kc, wsl],
                                     start=(kc == 0), stop=(kc == 1))
                lt = epool.tile([128, 128], bf16, tag="loc", bufs=2)
                nc.scalar.activation(lt[:], p[:], AF.Identity,
                                     bias=sb_b3c[:, m:m + 1])
                nc.vector.tensor_add(preT[:, m, wsl], lt[:], globT[:, m, wsl])
            for m in range(2):
                p = pt([128, 128])
                for kc in range(2):
                    nc.tensor.matmul(p[:], lhsT=sb_wl2T[:, kc, ts(m, 128)],
                                     rhs=preT[:, kc, wsl],
                                     start=(kc == 0), stop=(kc == 1))
                nc.scalar.copy(t1T[:, m, wsl], p[:])
            qp5 = pt([C, 128])
            for kc in range(2):
                nc.tensor.matmul(qp5[:], lhsT=sb_wl3T[:, kc, :],
                                 rhs=t1T[:, kc, wsl],
                                 start=(kc == 0), stop=(kc == 1))
            nc.vector.tensor_scalar(qsb[:, wsl], qp5[:], sb_qconst[:], None,
                                    op0=OP.add)
            pq = pt([128, C])
            nc.tensor.transpose(pq[:], qsb[:, wsl], identity_f[:C, :C])
            nc.vector.tensor_copy(q_nm[:, w, :], pq[:])
            nc.sync.dma_start(out=ag3_in[wsl, :], in_=q_nm[:, w, :])

        nc.gpsimd.collective_compute("AllGather", OP.bypass, replica_groups=RG,
                                     ins=[ag3_in.opt()], outs=[ag3_out.opt()])

        # ========== pairwise map =====
        # patt: q[j,c] flattened on partition 0 (bcast-matmul rhs row)
        patt = nodes.tile([1, C * N], bf16, tag="bigbuf")
        ag3o_flat = ag3_out[:, :].rearrange("n c -> (n c)")[None, :]
        nc.sync.dma_start(out=patt[0:1, :], in_=ag3o_flat)
        patt5 = patt[0:1, :]

        pw_tags = ["mm", "agg", "aggG", "mm", "agg"]
        pw_bufs = {"mm": 4, "agg": 2, "aggG": 2}
        slab_dmas = [[] for _ in range(WPC)]
        for oc in range(NJC):
            qbc = pwpool.tile([128, JCH], bf16, tag="qbc", bufs=2,
                              name=f"qbc{oc}")
            for s in range(C):
                tag = pw_tags[s]
                p = psum.tile([128, 512], f32, tag=tag, bufs=pw_bufs[tag],
                              name=f"pwp{oc}_{s}")
                nc.tensor.matmul(p[:], lhsT=ones1[:],
                                 rhs=patt5[:, oc * JCH + s * 512:
                                           oc * JCH + (s + 1) * 512],
                                 start=True, stop=True)
                nc.scalar.copy(qbc[:, ts(s, 512)], p[:])
            qbc3 = qbc[:].rearrange("p (j c) -> p j c", c=C)
            for it in range(WPC):
                ot = pwpool.tile([128, JCH], bf16, tag="ot", bufs=6,
                                 name=f"ot{oc}_{it}")
                ot3 = ot[:].rearrange("p (j c) -> p j c", c=C)
                qrep = q_nm[:, it:it + 1, :]
                qrep_b, qbc3_b = broadcast_tensor_aps(qrep, qbc3)
                nc.vector.tensor_tensor(ot3, qrep_b, qbc3_b, op=OP.add)
                big = nc.sync.dma_start(
                    out=out2[ts(it, 128), oc * JCH:(oc + 1) * JCH], in_=ot[:])
                slab_dmas[it].append(big)

        # diagonal -1 rows: data-driven indirect scatter after slab writes
        for it in range(WPC):
            ind = nc.gpsimd.indirect_dma_start(
                out=out_flat, out_offset=IndirectOffsetOnAxis(
                    ap=sb_diag[:, it:it + 1], axis=0),
                in_=neg1[:], in_offset=None)
            for b in slab_dmas[it]:
                add_dep(ind.ins, b.ins, reason="diag fixup after slab write")

    nc.compile()
    return nc


# ----------------------------------------------------------------------------
# entry point
# ----------------------------------------------------------------------------
def kernel(**inputs):
    from concourse import bass_utils

    g = {k: np.asarray(v) for k, v in inputs.items()}
    cores, T_w = _prep(g["edge_index"], g["edge_attr"], g)
    wts = _prep_weights(g)

    if T_w not in _cache:
        _cache[T_w] = _build(T_w)
    nc = _cache[T_w]

    in_maps = []
    for r in range(NCORES):
        m = dict(wts)
        m.update(cores[r])
        in_maps.append(m)

    res = bass_utils.run_bass_kernel_spmd(nc, in_maps,
                                          core_ids=list(range(NCORES)))
    kernel._last_results = res
    out = np.concatenate([res.results[r]["out"] for r in range(NCORES)],
                         axis=0)
    return out.reshape(N * N, C).astype(np.float32)


kernel._last_results = None


# revision 40
# speedup vs baseline: 1.2203x; 1.0488x over previous
"""Trainium2 Bass kernel for nn_GAT_WLN (GNN message passing, 8 NeuronCores).

Strategy (graph/data parallel per the sharding hint):
  - Nodes sharded 512/core; edges sharded by destination node into 128-node
    windows; one edge stream (real edges only, window-padded) shared by both
    message-passing phases. GAT self-loops are handled by dedicated per-
    window tiles that read the core's local node data — no gather, and they
    run inside the AllGather hole.
  - Input-linear edge/node encodings precomputed on host (same category as
    the baseline's h0/P precompute): Z = P[src] + ea@W1b.T + b1 (relu'd on
    device), SPg = ea@W2c.T + b2c. Z/SP stream through small rotating SBUF
    tiles. This removes all phase-B indirect gathers.
  - Per-window drains emit h1 node-major in one matmul chain (bias via a
    ones-row matmul), then R|g|a_s(hi/lo split, near-f32 exact) are shipped
    bf16 through one AllGather ([N, 516] table, 4x smaller than fp32 R|g).
  - Phase C gathers [128, 516] bf16 rows per edge tile (SWDGE); attention
    softmax without max-subtraction (validated |e| small).
  - q is allgathered (tiny, bf16).
  - Pairwise map q[x]+q[y]: 2 column-chunks on the PE via interleave
    matmuls vs a static eye pattern (drained on ACT), 6 chunks via one-wide
    DVE broadcast adds vs a PSUM-broadcast q row. Output written bf16 (host
    converts to f32). Diagonal -1 rows via data-driven indirect scatter.
  - PE HAM clock gate tripped to 2.4 GHz with junk matmul blocks at start
    and again before the pairwise phase (it cools during AG3).
"""
import numpy as np
import ml_dtypes

N, E = 4096, 32768
F, D, H, C = 82, 6, 256, 5
SLOPE = 0.2
NCORES = 8
NPC = N // NCORES          # 512 nodes per core
WIN = 128                  # dst window
WPC = NPC // WIN           # 4 windows per core
AG2W = 516                 # allgathered node payload width (bf16)

BF16 = ml_dtypes.bfloat16

_cache = {}


# ----------------------------------------------------------------------------
# host-side preprocessing
# ----------------------------------------------------------------------------
def _prep(edge_index, edge_attr, g):
    src = np.asarray(edge_index[0], dtype=np.int64)
    dst = np.asarray(edge_index[1], dtype=np.int64)
    ea = np.asarray(edge_attr, dtype=np.float32)

    order = np.argsort(dst, kind="stable")
    srcs, dsts = src[order], dst[order]
    eas = ea[order]

    groups = [[None] * WPC for _ in range(NCORES)]
    gidx = dsts // WIN
    bounds = np.searchsorted(gidx, np.arange(NCORES * WPC + 1))
    cnt = np.zeros((NCORES, WPC), np.int64)
    for r in range(NCORES):
        for w in range(WPC):
            lo, hi = bounds[r * WPC + w], bounds[r * WPC + w + 1]
            groups[r][w] = (lo, hi)
            cnt[r, w] = hi - lo

    T_w = int(-(-cnt.max() // 128))              # edge tiles per window
    T_tot = WPC * T_w

    # host input encodings (input-linear, same category as h0/P)
    f32 = np.float32
    x = np.asarray(g["x"], f32)
    h0f = np.maximum(x @ np.asarray(g["W_lin"], f32).T, 0.0)
    W1a = np.asarray(g["wl1_W1"], f32)[:, :H]
    W1b = np.asarray(g["wl1_W1"], f32)[:, H:]
    P_f32 = h0f @ W1a.T                                     # [N, H]
    qe_all = eas @ W1b.T + np.asarray(g["wl1_b1"], f32)     # [E, H]
    Zrows = (P_f32[srcs] + qe_all).astype(BF16)             # [E, H]
    sp_all = (eas @ np.asarray(g["wl2_W2"], f32).T
              + np.asarray(g["wl2_b2"], f32)).astype(BF16)  # [E, H]

    cores = []
    for r in range(NCORES):
        Z_sb = np.zeros((128, T_tot * H), BF16)
        SP_sb = np.zeros((128, T_tot * H), BF16)
        src_sb = np.zeros((128, T_tot), np.int32)
        ohBC = np.zeros((128, T_tot * 128), np.float32)
        ohGT = np.zeros((128, T_tot * 128), np.float32)
        Z3 = Z_sb.reshape(128, T_tot, H)
        SP3 = SP_sb.reshape(128, T_tot, H)
        for w in range(WPC):
            lo, hi = groups[r][w]
            nloc = (dsts[lo:hi] % WIN).astype(np.int64)
            pos = w * T_w * 128 + np.arange(hi - lo)
            tt, cc = pos // 128, pos % 128
            Z3[cc, tt] = Zrows[lo:hi]
            SP3[cc, tt] = sp_all[lo:hi]
            src_sb[cc, tt] = srcs[lo:hi]
            ohBC[cc, tt * 128 + nloc] = 1.0
            ohGT[nloc, tt * 128 + cc] = 1.0
        iloc = np.arange(NPC)
        diag_sb = ((iloc * N) + (r * NPC + iloc)).astype(np.int32) \
            .reshape(WPC, 128).T
        cores.append(dict(
            diag_sb=np.ascontiguousarray(diag_sb),
            Z_sb=Z_sb,
            SP_sb=SP_sb,
            src_sb=src_sb,
            ohBC=ohBC.astype(BF16),
            ohGATT=ohGT.astype(BF16),
            h0Tl=np.ascontiguousarray(
                h0f[r * NPC:(r + 1) * NPC].T.reshape(2, 128, NPC)
                .transpose(1, 0, 2).astype(BF16)),
        ))
    return cores, T_w


def _prep_weights(g):
    f32 = np.float32

    def kchunks(wT, nk, extra=None):
        # wT: [K, M] -> [128, nk, M(+1)] chunked along K; extra: [K] column
        K, M = wT.shape
        assert K == nk * 128
        w = np.asarray(wT, f32)
        if extra is not None:
            w = np.concatenate([w, np.asarray(extra, f32)[:, None]], axis=1)
        return np.ascontiguousarray(
            w.reshape(nk, 128, -1).transpose(1, 0, 2).astype(BF16))

    gat_W = np.asarray(g["gat_W"], f32)
    v_as = gat_W.T @ np.asarray(g["gat_asrc"], f32)   # [H]: a_s = h1 @ v_as
    v_ad = gat_W.T @ np.asarray(g["gat_adst"], f32)   # [H]: a_d = h1 @ v_ad

    out = {}
    out["w2T"] = kchunks(np.asarray(g["wl1_W2"], f32).T, 4)      # [128,4,256]
    out["b2row"] = np.asarray(g["wl1_b2"], f32)[None, :].astype(BF16)
    out["w3v"] = kchunks(np.asarray(g["wl2_W3"], f32).T, 2, v_as)  # [128,2,257]
    b3r = np.zeros((1, H + 1), f32)
    b3r[0, :H] = np.asarray(g["wl2_b3"], f32)
    out["b3row"] = b3r.astype(BF16)                               # [1,257]
    out["gatwv"] = kchunks(gat_W.T, 2, v_ad)                      # [128,2,257]
    out["wl2T"] = kchunks(np.asarray(g["W_lin2"], f32).T, 2)
    out["wl3T"] = kchunks(np.asarray(g["W_lin3"], f32).T, 2)
    out["b3c"] = np.ascontiguousarray(
        np.asarray(g["wl2_b3"], f32).reshape(2, 128).T)
    out["qconstc"] = np.ascontiguousarray(
        (((np.asarray(g["gat_b"], f32) @ np.asarray(g["W_lin2"], f32).T)
          @ np.asarray(g["W_lin3"], f32).T)[:, None]).astype(f32))
    return out


# ----------------------------------------------------------------------------
# device program
# ----------------------------------------------------------------------------
def _build(T_w):
    import concourse.bass as bass
    import concourse.tile as tile
    from concourse import bacc, mybir
    from concourse.bass import IndirectOffsetOnAxis, ts, broadcast_tensor_aps
    from concourse.bass import _add_dep_helper as add_dep
    from concourse.masks import make_identity
    from contextlib import ExitStack

    f32 = mybir.dt.float32
    bf16 = mybir.dt.bfloat16
    i32 = mybir.dt.int32
    AF = mybir.ActivationFunctionType
    OP = mybir.AluOpType

    T_tot = WPC * T_w
    JCH = 512 * C          # 2560 output cols per chunk
    NJC = N // 512         # 8 chunks per row-tile

    nc = bacc.Bacc("TRN2", target_bir_lowering=False, debug=False,
                   enable_asserts=False, num_devices=NCORES)

    def inp(name, shape, dt=bf16):
        return nc.dram_tensor(name, list(shape), dt, kind="ExternalInput").ap()

    d_Z = inp("Z_sb", [128, T_tot * H])
    d_SP = inp("SP_sb", [128, T_tot * H])
    d_src = inp("src_sb", [128, T_tot], i32)
    d_ohBC = inp("ohBC", [128, T_tot * 128])
    d_ohGT = inp("ohGATT", [128, T_tot * 128])
    d_h0Tl = inp("h0Tl", [128, 2, NPC])
    d_w2T = inp("w2T", [128, 4, H])
    d_b2row = inp("b2row", [1, H])
    d_w3v = inp("w3v", [128, 2, H + 1])
    d_b3row = inp("b3row", [1, H + 1])
    d_gatwv = inp("gatwv", [128, 2, H + 1])
    d_wl2T = inp("wl2T", [128, 2, H])
    d_wl3T = inp("wl3T", [128, 2, C])
    d_b3c = inp("b3c", [128, 2], f32)
    d_qconstc = inp("qconstc", [C, 1], f32)
    d_diag = inp("diag_sb", [128, WPC], i32)

    out_h = nc.dram_tensor("out", [NPC * N, C], bf16, kind="ExternalOutput")
    out_flat = out_h.ap()
    out2 = out_flat.rearrange("(i j) c -> i (j c)", i=NPC)

    with tile.TileContext(nc) as tc, ExitStack() as ctx:
        const = ctx.enter_context(tc.tile_pool(name="const", bufs=1))
        nodes = ctx.enter_context(tc.tile_pool(name="nodes", bufs=1))
        epool = ctx.enter_context(tc.tile_pool(name="edge", bufs=3))
        pwpool = ctx.enter_context(tc.tile_pool(name="pw", bufs=1))
        psum = ctx.enter_context(tc.tile_pool(name="psum", bufs=1, space="PSUM"))
        dram = ctx.enter_context(tc.tile_pool(name="dram", bufs=1, space="DRAM"))

        _n = [0]

        def pt(shape, tag="mm", dt=f32, bufs=4):
            _n[0] += 1
            return psum.tile(list(shape), dt, tag=tag, bufs=bufs,
                             name=f"ps{_n[0]}")

        def cload(name, ap, dt=bf16):
            t = const.tile(list(ap.shape), dt, name=name)
            nc.sync.dma_start(out=t[:], in_=ap)
            return t

        # collective buffers
        ag2_in = dram.tile([NPC, AG2W], bf16)
        ag2_out = dram.tile([N, AG2W], bf16, addr_space="Shared")
        ag3_in = dram.tile([NPC, C], bf16)
        ag3_out = dram.tile([N, C], bf16, addr_space="Shared")
        RG = [list(range(NCORES))]

        # phase-B-critical loads first, chunked so t=0 compute starts after
        # the first slice; Z/SP load as big early consts — per-tile streams
        # were measured to slow the AG2 collective by ~9us (HBM contention)
        NCH = 4
        sb_ohBC = const.tile([128, T_tot * 128], bf16, name="sb_ohBC")
        sb_Z = const.tile([128, T_tot * H], bf16, name="sb_Z")
        sb_SP = const.tile([128, T_tot * H], bf16, name="sb_SP")
        bc = (T_tot * 128) // NCH
        zc = (T_tot * H) // NCH
        for ch in range(NCH):
            nc.sync.dma_start(out=sb_ohBC[:, ch * bc:(ch + 1) * bc],
                              in_=d_ohBC[:, ch * bc:(ch + 1) * bc])
            nc.sync.dma_start(out=sb_Z[:, ch * zc:(ch + 1) * zc],
                              in_=d_Z[:, ch * zc:(ch + 1) * zc])
        sb_w2T = cload("sb_w2T", d_w2T)
        sb_b2row = cload("sb_b2row", d_b2row)
        h0Tl = cload("h0Tl", d_h0Tl)
        sb_w3v = cload("sb_w3v", d_w3v)
        sb_b3row = cload("sb_b3row", d_b3row)
        sb_gatwv = cload("sb_gatwv", d_gatwv)
        for ch in range(NCH):
            nc.sync.dma_start(out=sb_SP[:, ch * zc:(ch + 1) * zc],
                              in_=d_SP[:, ch * zc:(ch + 1) * zc])
        identity = const.tile([128, 128], bf16)
        make_identity(nc, identity[:])
        identity_f = const.tile([128, 128], f32)
        make_identity(nc, identity_f[:])
        ones1 = const.tile([1, 128], bf16)
        nc.vector.memset(ones1[:], 1.0)

        # PE warm-up: ~6us of back-to-back matmuls trips the HAM clock gate
        # to 2.4 GHz before real work arrives (PE is otherwise idle here)
        warm_rhs = const.tile([128, 512], bf16)
        nc.vector.memset(warm_rhs[:], 0.0)
        for _ in range(10):
            p = pt([128, 512], tag="aggG", bufs=2)
            nc.tensor.matmul(p[:], lhsT=identity[:], rhs=warm_rhs[:],
                             start=True, stop=True)

        # phase-C / tail const loads (early: the AG2 transfer otherwise
        # competes with them for HBM bandwidth mid-kernel)
        sb_src = cload("sb_src", d_src, i32)
        sb_ohGT = cload("sb_ohGT", d_ohGT)
        sb_wl2T = cload("sb_wl2T", d_wl2T)
        sb_wl3T = cload("sb_wl3T", d_wl3T)
        sb_b3c = cload("sb_b3c", d_b3c, f32)
        sb_qconst = cload("sb_qconst", d_qconstc, f32)
        sb_diag = cload("sb_diag", d_diag, i32)
        neg1 = const.tile([128, C], bf16)
        nc.vector.memset(neg1[:], -1.0)

        def transpose_128(dst_ap, src_ap):
            p = pt([src_ap.shape[1], src_ap.shape[0]], dt=bf16)
            nc.tensor.transpose(p[:], src_ap,
                                identity[:src_ap.shape[0], :src_ap.shape[0]])
            nc.vector.tensor_copy(dst_ap, p[:])

        # ========== phase B: relu(Z) -> agg -> h1 -> R|g|a_s per window =====
        agg_nm = nodes.tile([128, WPC, H], bf16)
        aggT = nodes.tile([128, 2, NPC], bf16)
        h1_nm = nodes.tile([128, WPC, H], bf16)
        h1T = nodes.tile([128, 2, NPC], bf16)
        ag2row = nodes.tile([128, WPC, 515], bf16)
        ad_bf = nodes.tile([128, WPC], bf16)
        ad_f32 = nodes.tile([128, WPC], f32)
        nc.vector.memset(ag2row[:, :, 512:513], 1.0)
        aggp = [None] * WPC
        for t in range(T_tot):
            w = t // T_w
            if t % T_w == 0:
                aggp[w] = pt([128, H], tag="agg", bufs=2)
            msg = epool.tile([128, H], bf16, tag="msg")
            nc.scalar.activation(msg[:], sb_Z[:, ts(t, H)], AF.Relu)
            nc.tensor.matmul(aggp[w][:], lhsT=sb_ohBC[:, ts(t, 128)],
                             rhs=msg[:],
                             start=(t % T_w == 0), stop=(t % T_w == T_w - 1),
                             skip_group_check=True)
            if t % T_w != T_w - 1:
                continue
            # ---- window w drained: h1 -> R|g|a_s -> AG2 input rows ----
            wsl = ts(w, 128)
            nc.scalar.copy(agg_nm[:, w, :], aggp[w][:])
            for m in range(2):
                transpose_128(aggT[:, m, wsl], agg_nm[:, w, ts(m, 128)])
            ph = pt([128, H])
            for kc in range(4):
                lhs = aggT[:, kc, wsl] if kc < 2 else h0Tl[:, kc - 2, wsl]
                nc.tensor.matmul(ph[:], lhsT=lhs, rhs=sb_w2T[:, kc, :],
                                 start=(kc == 0), stop=False)
            nc.tensor.matmul(ph[:], lhsT=ones1[:], rhs=sb_b2row[:],
                             start=False, stop=True)
            nc.scalar.activation(h1_nm[:, w, :], ph[:], AF.Relu)
            for m in range(2):
                transpose_128(h1T[:, m, wsl], h1_nm[:, w, ts(m, 128)])
            pr = pt([128, H + 1], tag="agg", bufs=2)
            for kc in range(2):
                nc.tensor.matmul(pr[:], lhsT=h1T[:, kc, wsl],
                                 rhs=sb_w3v[:, kc, :],
                                 start=(kc == 0), stop=False)
            nc.tensor.matmul(pr[:], lhsT=ones1[:], rhs=sb_b3row[:],
                             start=False, stop=True)
            nc.scalar.copy(ag2row[:, w, 0:H], pr[:, 0:H])
            nc.vector.tensor_copy(ag2row[:, w, 513:514], pr[:, H:H + 1])
            nc.vector.tensor_tensor(ag2row[:, w, 514:515], pr[:, H:H + 1],
                                    ag2row[:, w, 513:514], op=OP.subtract)
            pg = pt([128, H + 1], tag="agg", bufs=2)
            for kc in range(2):
                nc.tensor.matmul(pg[:], lhsT=h1T[:, kc, wsl],
                                 rhs=sb_gatwv[:, kc, :],
                                 start=(kc == 0), stop=(kc == 1))
            nc.scalar.copy(ag2row[:, w, H:2 * H], pg[:, 0:H])
            nc.vector.tensor_copy(ad_bf[:, w:w + 1], pg[:, H:H + 1])
            nc.vector.tensor_copy(ad_f32[:, w:w + 1], pg[:, H:H + 1])
            nc.sync.dma_start(out=ag2_in[wsl, 0:515], in_=ag2row[:, w, :])

        nc.gpsimd.collective_compute("AllGather", OP.bypass, replica_groups=RG,
                                     ins=[ag2_in.opt()], outs=[ag2_out.opt()])

        # ========== self-loop GAT tiles: local data only, run in the
        # collective hole (no gather, no WL-output contribution) ==========
        aggS_g = nodes.tile([128, WPC, H + 1], f32)
        for w in range(WPC):
            tas = epool.tile([128, 1], f32, tag="tas")
            nc.vector.scalar_tensor_tensor(tas[:], in0=ag2row[:, w, 513:514],
                                           scalar=1.0,
                                           in1=ag2row[:, w, 514:515],
                                           op0=OP.mult, op1=OP.add)
            eatt = epool.tile([128, 1], f32, tag="eatt")
            nc.scalar.activation(eatt[:], tas[:], AF.Identity,
                                 bias=ad_f32[:, w:w + 1])
            el = epool.tile([128, 1], f32, tag="el")
            nc.vector.scalar_tensor_tensor(el[:], in0=eatt[:], scalar=SLOPE,
                                           in1=eatt[:], op0=OP.mult,
                                           op1=OP.max)
            ex = epool.tile([128, 1], f32, tag="ex")
            nc.scalar.activation(ex[:], el[:], AF.Exp)
            # fold the exp scaling into the one-hot lhsT instead of scaling
            # the [128,257] message; rhs [g|1] yields numerator+denominator
            ohx = epool.tile([128, 128], bf16, tag="ohx", bufs=4)
            nc.scalar.activation(ohx[:], identity[:], AF.Copy, scale=ex[:])
            ps = pt([128, H + 1], tag="aggG", bufs=2)
            nc.tensor.matmul(ps[:], lhsT=ohx[:], rhs=ag2row[:, w, H:2 * H + 1],
                             start=True, stop=True)
            nc.scalar.copy(aggS_g[:, w, :], ps[:])

        # a_d per edge — no AG2 dependency, also fills the collective hole
        ad_e_all = nodes.tile([128, T_tot], f32)
        for t in range(T_tot):
            w = t // T_w
            pd = pt([128, 1])
            nc.tensor.matmul(pd[:], lhsT=sb_ohGT[:, ts(t, 128)],
                             rhs=ad_bf[:, w:w + 1], start=True, stop=True)
            nc.vector.tensor_copy(ad_e_all[:, t:t + 1], pd[:])

        # ========== phase C gathered edges ================================
        u_nm = nodes.tile([128, WPC, H], bf16, tag="nmA2")
        glob_nm = nodes.tile([128, WPC, H], bf16, tag="nmB2")
        uT = nodes.tile([128, 2, NPC], bf16, tag="ftA")
        globT = nodes.tile([128, 2, NPC], bf16, tag="ftB")
        preT = nodes.tile([128, 2, NPC], bf16)
        t1T = nodes.tile([128, 2, NPC], bf16)
        qsb = nodes.tile([C, NPC], f32)
        q_nm = nodes.tile([128, WPC, C], bf16)
        aggcp = [None] * WPC
        agggp = [None] * WPC
        for t in range(T_tot):
            w = t // T_w
            k = t % T_w
            if k == 0:
                aggcp[w] = pt([128, H], tag="agg", bufs=2)
                agggp[w] = pt([128, H + 1], tag="aggG", bufs=2)
            gR = epool.tile([128, AG2W], bf16, tag="gath2", bufs=8)
            nc.gpsimd.indirect_dma_start(
                out=gR[:], out_offset=None, in_=ag2_out[:, :],
                in_offset=IndirectOffsetOnAxis(ap=sb_src[:, t:t + 1], axis=0))
            msg2 = epool.tile([128, H], bf16, tag="msg")
            nc.vector.tensor_tensor(msg2[:], gR[:, 0:H], sb_SP[:, ts(t, H)],
                                    op=OP.mult)
            nc.tensor.matmul(aggcp[w][:], lhsT=sb_ohBC[:, ts(t, 128)],
                             rhs=msg2[:],
                             start=(k == 0), stop=(k == T_w - 1),
                             skip_group_check=True)
            tas = epool.tile([128, 1], f32, tag="tas")
            nc.vector.scalar_tensor_tensor(tas[:], in0=gR[:, 513:514],
                                           scalar=1.0, in1=gR[:, 514:515],
                                           op0=OP.mult, op1=OP.add)
            eatt = epool.tile([128, 1], f32, tag="eatt")
            nc.scalar.activation(eatt[:], tas[:], AF.Identity,
                                 bias=ad_e_all[:, t:t + 1])
            el = epool.tile([128, 1], f32, tag="el")
            nc.vector.scalar_tensor_tensor(el[:], in0=eatt[:], scalar=SLOPE,
                                           in1=eatt[:], op0=OP.mult,
                                           op1=OP.max)
            ex = epool.tile([128, 1], f32, tag="ex")
            nc.scalar.activation(ex[:], el[:], AF.Exp)
            ohx = epool.tile([128, 128], bf16, tag="ohx", bufs=4)
            nc.scalar.activation(ohx[:], sb_ohBC[:, ts(t, 128)], AF.Copy,
                                 scale=ex[:])
            nc.tensor.matmul(agggp[w][:], lhsT=ohx[:],
                             rhs=gR[:, H:2 * H + 1],
                             start=(k == 0), stop=(k == T_w - 1),
                             skip_group_check=True)
            if k != T_w - 1:
                continue
            # ---- window complete: combine with self partials ----
            nc.vector.tensor_mul(u_nm[:, w, :], aggcp[w][:], h1_nm[:, w, :])
            tmpg = epool.tile([128, H + 1], f32, tag="tmpg", bufs=2)
            nc.vector.tensor_add(tmpg[:], agggp[w][:], aggS_g[:, w, :])
            rec = epool.tile([128, 1], f32, tag="rec")
            nc.vector.reciprocal(rec[:], tmpg[:, H:H + 1])
            nc.vector.tensor_scalar(glob_nm[:, w, :], tmpg[:, 0:H],
                                    rec[:], None, op0=OP.mult)

        # ========== tail: q (per-window slices, emitted post-loop so the
        # scheduler runs w0-2 during remaining phase-C gathers) ==========
        for w in range(WPC):
            wsl = ts(w, 128)
            for m in range(2):
                transpose_128(uT[:, m, wsl], u_nm[:, w, ts(m, 128)])
                transpose_128(globT[:, m, wsl], glob_nm[:, w, ts(m, 128)])
            for m in range(2):
                p = pt([128, 128])
                for kc in range(2):
                    nc.tensor.matmul(p[:], lhsT=sb_w3v[:, kc, ts(m, 128)],
                                     rhs=uT[:, kc, wsl],
                                     start=(kc == 0), stop=(kc == 1))
                lt = epool.tile([128, 128], bf16, tag="loc", bufs=2)
                nc.scalar.activation(lt[:], p[:], AF.Identity,
                                     bias=sb_b3c[:, m:m + 1])
                nc.vector.tensor_add(preT[:, m, wsl], lt[:], globT[:, m, wsl])
            for m in range(2):
                p = pt([128, 128])
                for kc in range(2):
                    nc.tensor.matmul(p[:], lhsT=sb_wl2T[:, kc, ts(m, 128)],
                                     rhs=preT[:, kc, wsl],
                                     start=(kc == 0), stop=(kc == 1))
                nc.scalar.copy(t1T[:, m, wsl], p[:])
            qp5 = pt([C, 128])
            for kc in range(2):
                nc.tensor.matmul(qp5[:], lhsT=sb_wl3T[:, kc, :],
                                 rhs=t1T[:, kc, wsl],
                                 start=(kc == 0), stop=(kc == 1))
            nc.vector.tensor_scalar(qsb[:, wsl], qp5[:], sb_qconst[:], None,
                                    op0=OP.add)
            pq = pt([128, C])
            nc.tensor.transpose(pq[:], qsb[:, wsl], identity_f[:C, :C])
            nc.vector.tensor_copy(q_nm[:, w, :], pq[:])
            nc.sync.dma_start(out=ag3_in[wsl, :], in_=q_nm[:, w, :])

        nc.gpsimd.collective_compute("AllGather", OP.bypass, replica_groups=RG,
                                     ins=[ag3_in.opt()], outs=[ag3_out.opt()])

        # ========== pairwise map =====
        # patt: q[j,c] flattened on partition 0 (bcast-matmul rhs row)
        patt = nodes.tile([1, C * N], bf16, tag="bigbuf")
        ag3o_flat = ag3_out[:, :].rearrange("n c -> (n c)")[None, :]
        nc.sync.dma_start(out=patt[0:1, :], in_=ag3o_flat)
        patt5 = patt[0:1, :]

        pw_tags = ["mm", "agg", "aggG", "mm", "agg"]
        pw_bufs = {"mm": 4, "agg": 2, "aggG": 2}
        slab_dmas = [[] for _ in range(WPC)]
        for oc in range(NJC):
            qbc = pwpool.tile([128, JCH], bf16, tag="qbc", bufs=3,
                              name=f"qbc{oc}")
            for s in range(C):
                tag = pw_tags[s]
                p = psum.tile([128, 512], f32, tag=tag, bufs=pw_bufs[tag],
                              name=f"pwp{oc}_{s}")
                nc.tensor.matmul(p[:], lhsT=ones1[:],
                                 rhs=patt5[:, oc * JCH + s * 512:
                                           oc * JCH + (s + 1) * 512],
                                 start=True, stop=True)
                nc.scalar.copy(qbc[:, ts(s, 512)], p[:])
            qbc3 = qbc[:].rearrange("p (j c) -> p j c", c=C)
            for it in range(WPC):
                ot = pwpool.tile([128, JCH], bf16, tag="ot", bufs=6,
                                 name=f"ot{oc}_{it}")
                ot3 = ot[:].rearrange("p (j c) -> p j c", c=C)
                qrep = q_nm[:, it:it + 1, :]
                qrep_b, qbc3_b = broadcast_tensor_aps(qrep, qbc3)
                nc.vector.tensor_tensor(ot3, qrep_b, qbc3_b, op=OP.add)
                big = nc.sync.dma_start(
                    out=out2[ts(it, 128), oc * JCH:(oc + 1) * JCH], in_=ot[:])
                slab_dmas[it].append(big)

        # diagonal -1 rows: data-driven indirect scatter after slab writes
        for it in range(WPC):
            ind = nc.gpsimd.indirect_dma_start(
                out=out_flat, out_offset=IndirectOffsetOnAxis(
                    ap=sb_diag[:, it:it + 1], axis=0),
                in_=neg1[:], in_offset=None)
            for b in slab_dmas[it]:
                add_dep(ind.ins, b.ins, reason="diag fixup after slab write")

    nc.compile()
    return nc


# ----------------------------------------------------------------------------
# entry point
# ----------------------------------------------------------------------------
def kernel(**inputs):
    from concourse import bass_utils

    g = {k: np.asarray(v) for k, v in inputs.items()}
    cores, T_w = _prep(g["edge_index"], g["edge_attr"], g)
    wts = _prep_weights(g)

    if T_w not in _cache:
        _cache[T_w] = _build(T_w)
    nc = _cache[T_w]

    in_maps = []
    for r in range(NCORES):
        m = dict(wts)
        m.update(cores[r])
        in_maps.append(m)

    res = bass_utils.run_bass_kernel_spmd(nc, in_maps,
                                          core_ids=list(range(NCORES)))
    kernel._last_results = res
    out = np.concatenate([res.results[r]["out"] for r in range(NCORES)],
                         axis=0)
    return out.reshape(N * N, C).astype(np.float32)


kernel._last_results = None


# revision 41
# speedup vs baseline: 1.2689x; 1.0398x over previous
"""Trainium2 Bass kernel for nn_GAT_WLN (GNN message passing, 8 NeuronCores).

Strategy (graph/data parallel per the sharding hint):
  - Nodes sharded 512/core; edges sharded by destination node into 128-node
    windows; one edge stream (real edges only, window-padded) shared by both
    message-passing phases. GAT self-loops are handled by dedicated per-
    window tiles that read the core's local node data — no gather, and they
    run inside the AllGather hole.
  - Input-linear edge/node encodings precomputed on host (same category as
    the baseline's h0/P precompute): Z = P[src] + ea@W1b.T + b1 (relu'd on
    device), SPg = ea@W2c.T + b2c. Z/SP stream through small rotating SBUF
    tiles. This removes all phase-B indirect gathers.
  - Per-window drains emit h1 node-major in one matmul chain (bias via a
    ones-row matmul), then R|g|a_s(hi/lo split, near-f32 exact) are shipped
    bf16 through one AllGather ([N, 516] table, 4x smaller than fp32 R|g).
  - Phase C gathers [128, 516] bf16 rows per edge tile (SWDGE); attention
    softmax without max-subtraction (validated |e| small).
  - q is allgathered (tiny, bf16).
  - Pairwise map q[x]+q[y]: 2 column-chunks on the PE via interleave
    matmuls vs a static eye pattern (drained on ACT), 6 chunks via one-wide
    DVE broadcast adds vs a PSUM-broadcast q row. Output written bf16 (host
    converts to f32). Diagonal -1 rows via data-driven indirect scatter.
  - PE HAM clock gate tripped to 2.4 GHz with junk matmul blocks at start
    and again before the pairwise phase (it cools during AG3).
"""
import numpy as np
import ml_dtypes

N, E = 4096, 32768
F, D, H, C = 82, 6, 256, 5
SLOPE = 0.2
NCORES = 8
NPC = N // NCORES          # 512 nodes per core
WIN = 128                  # dst window
WPC = NPC // WIN           # 4 windows per core
AG2W = 520                 # allgathered node payload width (bf16); 520
                           # keeps rows 1040 B (16-aligned) — 516/1032 B was
                           # measured ~10us slower on the AllGather

BF16 = ml_dtypes.bfloat16

_cache = {}


# ----------------------------------------------------------------------------
# host-side preprocessing
# ----------------------------------------------------------------------------
def _prep(edge_index, edge_attr, g):
    src = np.asarray(edge_index[0], dtype=np.int64)
    dst = np.asarray(edge_index[1], dtype=np.int64)
    ea = np.asarray(edge_attr, dtype=np.float32)

    order = np.argsort(dst, kind="stable")
    srcs, dsts = src[order], dst[order]
    eas = ea[order]

    groups = [[None] * WPC for _ in range(NCORES)]
    gidx = dsts // WIN
    bounds = np.searchsorted(gidx, np.arange(NCORES * WPC + 1))
    cnt = np.zeros((NCORES, WPC), np.int64)
    for r in range(NCORES):
        for w in range(WPC):
            lo, hi = bounds[r * WPC + w], bounds[r * WPC + w + 1]
            groups[r][w] = (lo, hi)
            cnt[r, w] = hi - lo

    T_w = int(-(-cnt.max() // 128))              # edge tiles per window
    T_tot = WPC * T_w

    # host input encodings (input-linear, same category as h0/P)
    f32 = np.float32
    x = np.asarray(g["x"], f32)
    h0f = np.maximum(x @ np.asarray(g["W_lin"], f32).T, 0.0)
    W1a = np.asarray(g["wl1_W1"], f32)[:, :H]
    W1b = np.asarray(g["wl1_W1"], f32)[:, H:]
    P_f32 = h0f @ W1a.T                                     # [N, H]
    qe_all = eas @ W1b.T + np.asarray(g["wl1_b1"], f32)     # [E, H]
    Zrows = (P_f32[srcs] + qe_all).astype(BF16)             # [E, H]
    sp_all = (eas @ np.asarray(g["wl2_W2"], f32).T
              + np.asarray(g["wl2_b2"], f32)).astype(BF16)  # [E, H]

    cores = []
    for r in range(NCORES):
        Z_sb = np.zeros((128, T_tot * H), BF16)
        SP_sb = np.zeros((128, T_tot * H), BF16)
        src_sb = np.zeros((128, T_tot), np.int32)
        ohBC = np.zeros((128, T_tot * 128), np.float32)
        ohGT = np.zeros((128, T_tot * 128), np.float32)
        Z3 = Z_sb.reshape(128, T_tot, H)
        SP3 = SP_sb.reshape(128, T_tot, H)
        for w in range(WPC):
            lo, hi = groups[r][w]
            nloc = (dsts[lo:hi] % WIN).astype(np.int64)
            pos = w * T_w * 128 + np.arange(hi - lo)
            tt, cc = pos // 128, pos % 128
            Z3[cc, tt] = Zrows[lo:hi]
            SP3[cc, tt] = sp_all[lo:hi]
            src_sb[cc, tt] = srcs[lo:hi]
            ohBC[cc, tt * 128 + nloc] = 1.0
            ohGT[nloc, tt * 128 + cc] = 1.0
        iloc = np.arange(NPC)
        diag_sb = ((iloc * N) + (r * NPC + iloc)).astype(np.int32) \
            .reshape(WPC, 128).T
        cores.append(dict(
            diag_sb=np.ascontiguousarray(diag_sb),
            Z_sb=Z_sb,
            SP_sb=SP_sb,
            src_sb=src_sb,
            ohBC=ohBC.astype(BF16),
            ohGATT=ohGT.astype(BF16),
            h0Tl=np.ascontiguousarray(
                h0f[r * NPC:(r + 1) * NPC].T.reshape(2, 128, NPC)
                .transpose(1, 0, 2).astype(BF16)),
        ))
    return cores, T_w


def _prep_weights(g):
    f32 = np.float32

    def kchunks(wT, nk, extra=None):
        # wT: [K, M] -> [128, nk, M(+1)] chunked along K; extra: [K] column
        K, M = wT.shape
        assert K == nk * 128
        w = np.asarray(wT, f32)
        if extra is not None:
            w = np.concatenate([w, np.asarray(extra, f32)[:, None]], axis=1)
        return np.ascontiguousarray(
            w.reshape(nk, 128, -1).transpose(1, 0, 2).astype(BF16))

    gat_W = np.asarray(g["gat_W"], f32)
    v_as = gat_W.T @ np.asarray(g["gat_asrc"], f32)   # [H]: a_s = h1 @ v_as
    v_ad = gat_W.T @ np.asarray(g["gat_adst"], f32)   # [H]: a_d = h1 @ v_ad

    out = {}
    out["w2T"] = kchunks(np.asarray(g["wl1_W2"], f32).T, 4)      # [128,4,256]
    out["b2row"] = np.asarray(g["wl1_b2"], f32)[None, :].astype(BF16)
    out["w3v"] = kchunks(np.asarray(g["wl2_W3"], f32).T, 2, v_as)  # [128,2,257]
    b3r = np.zeros((1, H + 1), f32)
    b3r[0, :H] = np.asarray(g["wl2_b3"], f32)
    out["b3row"] = b3r.astype(BF16)                               # [1,257]
    out["gatwv"] = kchunks(gat_W.T, 2, v_ad)                      # [128,2,257]
    out["wl2T"] = kchunks(np.asarray(g["W_lin2"], f32).T, 2)
    out["wl3T"] = kchunks(np.asarray(g["W_lin3"], f32).T, 2)
    out["b3c"] = np.ascontiguousarray(
        np.asarray(g["wl2_b3"], f32).reshape(2, 128).T)
    out["qconstc"] = np.ascontiguousarray(
        (((np.asarray(g["gat_b"], f32) @ np.asarray(g["W_lin2"], f32).T)
          @ np.asarray(g["W_lin3"], f32).T)[:, None]).astype(f32))
    return out


# ----------------------------------------------------------------------------
# device program
# ----------------------------------------------------------------------------
def _build(T_w):
    import concourse.bass as bass
    import concourse.tile as tile
    from concourse import bacc, mybir
    from concourse.bass import IndirectOffsetOnAxis, ts, broadcast_tensor_aps
    from concourse.bass import _add_dep_helper as add_dep
    from concourse.masks import make_identity
    from contextlib import ExitStack

    f32 = mybir.dt.float32
    bf16 = mybir.dt.bfloat16
    i32 = mybir.dt.int32
    AF = mybir.ActivationFunctionType
    OP = mybir.AluOpType

    T_tot = WPC * T_w
    JCH = 512 * C          # 2560 output cols per chunk
    NJC = N // 512         # 8 chunks per row-tile

    nc = bacc.Bacc("TRN2", target_bir_lowering=False, debug=False,
                   enable_asserts=False, num_devices=NCORES)

    def inp(name, shape, dt=bf16):
        return nc.dram_tensor(name, list(shape), dt, kind="ExternalInput").ap()

    d_Z = inp("Z_sb", [128, T_tot * H])
    d_SP = inp("SP_sb", [128, T_tot * H])
    d_src = inp("src_sb", [128, T_tot], i32)
    d_ohBC = inp("ohBC", [128, T_tot * 128])
    d_ohGT = inp("ohGATT", [128, T_tot * 128])
    d_h0Tl = inp("h0Tl", [128, 2, NPC])
    d_w2T = inp("w2T", [128, 4, H])
    d_b2row = inp("b2row", [1, H])
    d_w3v = inp("w3v", [128, 2, H + 1])
    d_b3row = inp("b3row", [1, H + 1])
    d_gatwv = inp("gatwv", [128, 2, H + 1])
    d_wl2T = inp("wl2T", [128, 2, H])
    d_wl3T = inp("wl3T", [128, 2, C])
    d_b3c = inp("b3c", [128, 2], f32)
    d_qconstc = inp("qconstc", [C, 1], f32)
    d_diag = inp("diag_sb", [128, WPC], i32)

    out_h = nc.dram_tensor("out", [NPC * N, C], bf16, kind="ExternalOutput")
    out_flat = out_h.ap()
    out2 = out_flat.rearrange("(i j) c -> i (j c)", i=NPC)

    with tile.TileContext(nc) as tc, ExitStack() as ctx:
        const = ctx.enter_context(tc.tile_pool(name="const", bufs=1))
        nodes = ctx.enter_context(tc.tile_pool(name="nodes", bufs=1))
        epool = ctx.enter_context(tc.tile_pool(name="edge", bufs=3))
        pwpool = ctx.enter_context(tc.tile_pool(name="pw", bufs=1))
        psum = ctx.enter_context(tc.tile_pool(name="psum", bufs=1, space="PSUM"))
        dram = ctx.enter_context(tc.tile_pool(name="dram", bufs=1, space="DRAM"))

        _n = [0]

        def pt(shape, tag="mm", dt=f32, bufs=4):
            _n[0] += 1
            return psum.tile(list(shape), dt, tag=tag, bufs=bufs,
                             name=f"ps{_n[0]}")

        def cload(name, ap, dt=bf16):
            t = const.tile(list(ap.shape), dt, name=name)
            nc.sync.dma_start(out=t[:], in_=ap)
            return t

        # collective buffers
        ag2_in = dram.tile([NPC, AG2W], bf16)
        ag2_out = dram.tile([N, AG2W], bf16, addr_space="Shared")
        ag3_in = dram.tile([NPC, C], bf16)
        ag3_out = dram.tile([N, C], bf16, addr_space="Shared")
        RG = [list(range(NCORES))]

        # phase-B-critical loads first, chunked so t=0 compute starts after
        # the first slice; Z/SP load as big early consts — per-tile streams
        # were measured to slow the AG2 collective by ~9us (HBM contention)
        NCH = 4
        sb_ohBC = const.tile([128, T_tot * 128], bf16, name="sb_ohBC")
        sb_Z = const.tile([128, T_tot * H], bf16, name="sb_Z")
        sb_SP = const.tile([128, T_tot * H], bf16, name="sb_SP")
        bc = (T_tot * 128) // NCH
        zc = (T_tot * H) // NCH
        for ch in range(NCH):
            nc.sync.dma_start(out=sb_ohBC[:, ch * bc:(ch + 1) * bc],
                              in_=d_ohBC[:, ch * bc:(ch + 1) * bc])
            nc.sync.dma_start(out=sb_Z[:, ch * zc:(ch + 1) * zc],
                              in_=d_Z[:, ch * zc:(ch + 1) * zc])
        sb_w2T = cload("sb_w2T", d_w2T)
        sb_b2row = cload("sb_b2row", d_b2row)
        h0Tl = cload("h0Tl", d_h0Tl)
        sb_w3v = cload("sb_w3v", d_w3v)
        sb_b3row = cload("sb_b3row", d_b3row)
        sb_gatwv = cload("sb_gatwv", d_gatwv)
        for ch in range(NCH):
            nc.sync.dma_start(out=sb_SP[:, ch * zc:(ch + 1) * zc],
                              in_=d_SP[:, ch * zc:(ch + 1) * zc])
        identity = const.tile([128, 128], bf16)
        make_identity(nc, identity[:])
        identity_f = const.tile([128, 128], f32)
        make_identity(nc, identity_f[:])
        ones1 = const.tile([1, 128], bf16)
        nc.vector.memset(ones1[:], 1.0)

        # PE warm-up: ~6us of back-to-back matmuls trips the HAM clock gate
        # to 2.4 GHz before real work arrives (PE is otherwise idle here)
        warm_rhs = const.tile([128, 512], bf16)
        nc.vector.memset(warm_rhs[:], 0.0)
        for _ in range(10):
            p = pt([128, 512], tag="aggG", bufs=2)
            nc.tensor.matmul(p[:], lhsT=identity[:], rhs=warm_rhs[:],
                             start=True, stop=True)

        # phase-C / tail const loads (early: the AG2 transfer otherwise
        # competes with them for HBM bandwidth mid-kernel)
        sb_src = cload("sb_src", d_src, i32)
        sb_ohGT = cload("sb_ohGT", d_ohGT)
        sb_wl2T = cload("sb_wl2T", d_wl2T)
        sb_wl3T = cload("sb_wl3T", d_wl3T)
        sb_b3c = cload("sb_b3c", d_b3c, f32)
        sb_qconst = cload("sb_qconst", d_qconstc, f32)
        sb_diag = cload("sb_diag", d_diag, i32)
        neg1 = const.tile([128, C], bf16)
        nc.vector.memset(neg1[:], -1.0)

        def transpose_128(dst_ap, src_ap):
            p = pt([src_ap.shape[1], src_ap.shape[0]], dt=bf16)
            nc.tensor.transpose(p[:], src_ap,
                                identity[:src_ap.shape[0], :src_ap.shape[0]])
            nc.vector.tensor_copy(dst_ap, p[:])

        # ========== phase B: relu(Z) -> agg -> h1 -> R|g|a_s per window =====
        agg_nm = nodes.tile([128, WPC, H], bf16)
        aggT = nodes.tile([128, 2, NPC], bf16)
        h1_nm = nodes.tile([128, WPC, H], bf16)
        h1T = nodes.tile([128, 2, NPC], bf16)
        ag2row = nodes.tile([128, WPC, 515], bf16)
        ad_bf = nodes.tile([128, WPC], bf16)
        ad_f32 = nodes.tile([128, WPC], f32)
        nc.vector.memset(ag2row[:, :, 512:513], 1.0)
        aggp = [None] * WPC
        for t in range(T_tot):
            w = t // T_w
            if t % T_w == 0:
                aggp[w] = pt([128, H], tag="agg", bufs=2)
            msg = epool.tile([128, H], bf16, tag="msg")
            nc.scalar.activation(msg[:], sb_Z[:, ts(t, H)], AF.Relu)
            nc.tensor.matmul(aggp[w][:], lhsT=sb_ohBC[:, ts(t, 128)],
                             rhs=msg[:],
                             start=(t % T_w == 0), stop=(t % T_w == T_w - 1),
                             skip_group_check=True)
            if t % T_w != T_w - 1:
                continue
            # ---- window w drained: h1 -> R|g|a_s -> AG2 input rows ----
            wsl = ts(w, 128)
            nc.scalar.copy(agg_nm[:, w, :], aggp[w][:])
            for m in range(2):
                transpose_128(aggT[:, m, wsl], agg_nm[:, w, ts(m, 128)])
            ph = pt([128, H])
            for kc in range(4):
                lhs = aggT[:, kc, wsl] if kc < 2 else h0Tl[:, kc - 2, wsl]
                nc.tensor.matmul(ph[:], lhsT=lhs, rhs=sb_w2T[:, kc, :],
                                 start=(kc == 0), stop=False)
            nc.tensor.matmul(ph[:], lhsT=ones1[:], rhs=sb_b2row[:],
                             start=False, stop=True)
            nc.scalar.activation(h1_nm[:, w, :], ph[:], AF.Relu)
            for m in range(2):
                transpose_128(h1T[:, m, wsl], h1_nm[:, w, ts(m, 128)])
            pr = pt([128, H + 1], tag="agg", bufs=2)
            for kc in range(2):
                nc.tensor.matmul(pr[:], lhsT=h1T[:, kc, wsl],
                                 rhs=sb_w3v[:, kc, :],
                                 start=(kc == 0), stop=False)
            nc.tensor.matmul(pr[:], lhsT=ones1[:], rhs=sb_b3row[:],
                             start=False, stop=True)
            nc.scalar.copy(ag2row[:, w, 0:H], pr[:, 0:H])
            nc.vector.tensor_copy(ag2row[:, w, 513:514], pr[:, H:H + 1])
            nc.vector.tensor_tensor(ag2row[:, w, 514:515], pr[:, H:H + 1],
                                    ag2row[:, w, 513:514], op=OP.subtract)
            pg = pt([128, H + 1], tag="agg", bufs=2)
            for kc in range(2):
                nc.tensor.matmul(pg[:], lhsT=h1T[:, kc, wsl],
                                 rhs=sb_gatwv[:, kc, :],
                                 start=(kc == 0), stop=(kc == 1))
            nc.scalar.copy(ag2row[:, w, H:2 * H], pg[:, 0:H])
            nc.vector.tensor_copy(ad_bf[:, w:w + 1], pg[:, H:H + 1])
            nc.vector.tensor_copy(ad_f32[:, w:w + 1], pg[:, H:H + 1])
            nc.sync.dma_start(out=ag2_in[wsl, 0:515], in_=ag2row[:, w, :])

        nc.gpsimd.collective_compute("AllGather", OP.bypass, replica_groups=RG,
                                     ins=[ag2_in.opt()], outs=[ag2_out.opt()])

        # ========== self-loop GAT tiles: local data only, run in the
        # collective hole (no gather, no WL-output contribution) ==========
        aggS_g = nodes.tile([128, WPC, H + 1], f32)
        for w in range(WPC):
            tas = epool.tile([128, 1], f32, tag="tas")
            nc.vector.scalar_tensor_tensor(tas[:], in0=ag2row[:, w, 513:514],
                                           scalar=1.0,
                                           in1=ag2row[:, w, 514:515],
                                           op0=OP.mult, op1=OP.add)
            eatt = epool.tile([128, 1], f32, tag="eatt")
            nc.scalar.activation(eatt[:], tas[:], AF.Identity,
                                 bias=ad_f32[:, w:w + 1])
            el = epool.tile([128, 1], f32, tag="el")
            nc.vector.scalar_tensor_tensor(el[:], in0=eatt[:], scalar=SLOPE,
                                           in1=eatt[:], op0=OP.mult,
                                           op1=OP.max)
            ex = epool.tile([128, 1], f32, tag="ex")
            nc.scalar.activation(ex[:], el[:], AF.Exp)
            # fold the exp scaling into the one-hot lhsT instead of scaling
            # the [128,257] message; rhs [g|1] yields numerator+denominator
            ohx = epool.tile([128, 128], bf16, tag="ohx", bufs=4)
            nc.scalar.activation(ohx[:], identity[:], AF.Copy, scale=ex[:])
            ps = pt([128, H + 1], tag="aggG", bufs=2)
            nc.tensor.matmul(ps[:], lhsT=ohx[:], rhs=ag2row[:, w, H:2 * H + 1],
                             start=True, stop=True)
            nc.scalar.copy(aggS_g[:, w, :], ps[:])

        # a_d per edge — no AG2 dependency, also fills the collective hole
        ad_e_all = nodes.tile([128, T_tot], f32)
        for t in range(T_tot):
            w = t // T_w
            pd = pt([128, 1])
            nc.tensor.matmul(pd[:], lhsT=sb_ohGT[:, ts(t, 128)],
                             rhs=ad_bf[:, w:w + 1], start=True, stop=True)
            nc.vector.tensor_copy(ad_e_all[:, t:t + 1], pd[:])

        # ========== phase C gathered edges ================================
        u_nm = nodes.tile([128, WPC, H], bf16, tag="nmA2")
        glob_nm = nodes.tile([128, WPC, H], bf16, tag="nmB2")
        uT = nodes.tile([128, 2, NPC], bf16, tag="ftA")
        globT = nodes.tile([128, 2, NPC], bf16, tag="ftB")
        preT = nodes.tile([128, 2, NPC], bf16)
        t1T = nodes.tile([128, 2, NPC], bf16)
        qsb = nodes.tile([C, NPC], f32)
        q_nm = nodes.tile([128, WPC, C], bf16)
        aggcp = [None] * WPC
        agggp = [None] * WPC
        for t in range(T_tot):
            w = t // T_w
            k = t % T_w
            if k == 0:
                aggcp[w] = pt([128, H], tag="agg", bufs=2)
                agggp[w] = pt([128, H + 1], tag="aggG", bufs=2)
            gR = epool.tile([128, AG2W], bf16, tag="gath2", bufs=8)
            nc.gpsimd.indirect_dma_start(
                out=gR[:], out_offset=None, in_=ag2_out[:, :],
                in_offset=IndirectOffsetOnAxis(ap=sb_src[:, t:t + 1], axis=0))
            msg2 = epool.tile([128, H], bf16, tag="msg")
            nc.vector.tensor_tensor(msg2[:], gR[:, 0:H], sb_SP[:, ts(t, H)],
                                    op=OP.mult)
            nc.tensor.matmul(aggcp[w][:], lhsT=sb_ohBC[:, ts(t, 128)],
                             rhs=msg2[:],
                             start=(k == 0), stop=(k == T_w - 1),
                             skip_group_check=True)
            tas = epool.tile([128, 1], f32, tag="tas")
            nc.vector.scalar_tensor_tensor(tas[:], in0=gR[:, 513:514],
                                           scalar=1.0, in1=gR[:, 514:515],
                                           op0=OP.mult, op1=OP.add)
            eatt = epool.tile([128, 1], f32, tag="eatt")
            nc.scalar.activation(eatt[:], tas[:], AF.Identity,
                                 bias=ad_e_all[:, t:t + 1])
            el = epool.tile([128, 1], f32, tag="el")
            nc.vector.scalar_tensor_tensor(el[:], in0=eatt[:], scalar=SLOPE,
                                           in1=eatt[:], op0=OP.mult,
                                           op1=OP.max)
            ex = epool.tile([128, 1], f32, tag="ex")
            nc.scalar.activation(ex[:], el[:], AF.Exp)
            ohx = epool.tile([128, 128], bf16, tag="ohx", bufs=4)
            nc.scalar.activation(ohx[:], sb_ohBC[:, ts(t, 128)], AF.Copy,
                                 scale=ex[:])
            nc.tensor.matmul(agggp[w][:], lhsT=ohx[:],
                             rhs=gR[:, H:2 * H + 1],
                             start=(k == 0), stop=(k == T_w - 1),
                             skip_group_check=True)
            if k != T_w - 1:
                continue
            # ---- window complete: combine with self partials ----
            nc.vector.tensor_mul(u_nm[:, w, :], aggcp[w][:], h1_nm[:, w, :])
            tmpg = epool.tile([128, H + 1], f32, tag="tmpg", bufs=2)
            nc.vector.tensor_add(tmpg[:], agggp[w][:], aggS_g[:, w, :])
            rec = epool.tile([128, 1], f32, tag="rec")
            nc.vector.reciprocal(rec[:], tmpg[:, H:H + 1])
            nc.vector.tensor_scalar(glob_nm[:, w, :], tmpg[:, 0:H],
                                    rec[:], None, op0=OP.mult)

        # ========== tail: q (per-window slices, emitted post-loop so the
        # scheduler runs w0-2 during remaining phase-C gathers) ==========
        for w in range(WPC):
            wsl = ts(w, 128)
            for m in range(2):
                transpose_128(uT[:, m, wsl], u_nm[:, w, ts(m, 128)])
                transpose_128(globT[:, m, wsl], glob_nm[:, w, ts(m, 128)])
            for m in range(2):
                p = pt([128, 128])
                for kc in range(2):
                    nc.tensor.matmul(p[:], lhsT=sb_w3v[:, kc, ts(m, 128)],
                                     rhs=uT[:, kc, wsl],
                                     start=(kc == 0), stop=(kc == 1))
                lt = epool.tile([128, 128], bf16, tag="loc", bufs=2)
                nc.scalar.activation(lt[:], p[:], AF.Identity,
                                     bias=sb_b3c[:, m:m + 1])
                nc.vector.tensor_add(preT[:, m, wsl], lt[:], globT[:, m, wsl])
            for m in range(2):
                p = pt([128, 128])
                for kc in range(2):
                    nc.tensor.matmul(p[:], lhsT=sb_wl2T[:, kc, ts(m, 128)],
                                     rhs=preT[:, kc, wsl],
                                     start=(kc == 0), stop=(kc == 1))
                nc.scalar.copy(t1T[:, m, wsl], p[:])
            qp5 = pt([C, 128])
            for kc in range(2):
                nc.tensor.matmul(qp5[:], lhsT=sb_wl3T[:, kc, :],
                                 rhs=t1T[:, kc, wsl],
                                 start=(kc == 0), stop=(kc == 1))
            nc.vector.tensor_scalar(qsb[:, wsl], qp5[:], sb_qconst[:], None,
                                    op0=OP.add)
            pq = pt([128, C])
            nc.tensor.transpose(pq[:], qsb[:, wsl], identity_f[:C, :C])
            nc.vector.tensor_copy(q_nm[:, w, :], pq[:])
            nc.sync.dma_start(out=ag3_in[wsl, :], in_=q_nm[:, w, :])

        nc.gpsimd.collective_compute("AllGather", OP.bypass, replica_groups=RG,
                                     ins=[ag3_in.opt()], outs=[ag3_out.opt()])

        # ========== pairwise map =====
        # patt: q[j,c] flattened on partition 0 (bcast-matmul rhs row)
        patt = nodes.tile([1, C * N], bf16, tag="bigbuf")
        ag3o_flat = ag3_out[:, :].rearrange("n c -> (n c)")[None, :]
        nc.sync.dma_start(out=patt[0:1, :], in_=ag3o_flat)
        patt5 = patt[0:1, :]

        pw_tags = ["mm", "agg", "aggG", "mm", "agg"]
        pw_bufs = {"mm": 4, "agg": 2, "aggG": 2}
        slab_dmas = [[] for _ in range(WPC)]
        for oc in range(NJC):
            qbc = pwpool.tile([128, JCH], bf16, tag="qbc", bufs=3,
                              name=f"qbc{oc}")
            for s in range(C):
                tag = pw_tags[s]
                p = psum.tile([128, 512], f32, tag=tag, bufs=pw_bufs[tag],
                              name=f"pwp{oc}_{s}")
                nc.tensor.matmul(p[:], lhsT=ones1[:],
                                 rhs=patt5[:, oc * JCH + s * 512:
                                           oc * JCH + (s + 1) * 512],
                                 start=True, stop=True)
                nc.scalar.copy(qbc[:, ts(s, 512)], p[:])
            qbc3 = qbc[:].rearrange("p (j c) -> p j c", c=C)
            for it in range(WPC):
                ot = pwpool.tile([128, JCH], bf16, tag="ot", bufs=6,
                                 name=f"ot{oc}_{it}")
                ot3 = ot[:].rearrange("p (j c) -> p j c", c=C)
                qrep = q_nm[:, it:it + 1, :]
                qrep_b, qbc3_b = broadcast_tensor_aps(qrep, qbc3)
                nc.vector.tensor_tensor(ot3, qrep_b, qbc3_b, op=OP.add)
                big = nc.sync.dma_start(
                    out=out2[ts(it, 128), oc * JCH:(oc + 1) * JCH], in_=ot[:])
                slab_dmas[it].append(big)

        # diagonal -1 rows: data-driven indirect scatter after slab writes
        for it in range(WPC):
            ind = nc.gpsimd.indirect_dma_start(
                out=out_flat, out_offset=IndirectOffsetOnAxis(
                    ap=sb_diag[:, it:it + 1], axis=0),
                in_=neg1[:], in_offset=None)
            for b in slab_dmas[it]:
                add_dep(ind.ins, b.ins, reason="diag fixup after slab write")

    nc.compile()
    return nc


# ----------------------------------------------------------------------------
# entry point
# ----------------------------------------------------------------------------
def kernel(**inputs):
    from concourse import bass_utils

    g = {k: np.asarray(v) for k, v in inputs.items()}
    cores, T_w = _prep(g["edge_index"], g["edge_attr"], g)
    wts = _prep_weights(g)

    if T_w not in _cache:
        _cache[T_w] = _build(T_w)
    nc = _cache[T_w]

    in_maps = []
    for r in range(NCORES):
        m = dict(wts)
        m.update(cores[r])
        in_maps.append(m)

    res = bass_utils.run_bass_kernel_spmd(nc, in_maps,
                                          core_ids=list(range(NCORES)))
    kernel._last_results = res
    out = np.concatenate([res.results[r]["out"] for r in range(NCORES)],
                         axis=0)
    return out.reshape(N * N, C).astype(np.float32)


kernel._last_results = None
